# revision 2
# baseline (speedup 1.0000x reference)
"""Trainium2 Bass kernel for NetTGCN (gnn_message_passing) — v3 (fp8 SpMM).

Structure follows v2 (column-major SpMM, 2 batch chains per conv, DRAM
AllGather per tap), with the Chebyshev SpMM switched to fp8 DoubleRow:

 * Lhat = -D M D with D = diag(dinv) and M the integer edge-multiplicity
   matrix.  M's entries (small ints) are EXACT in fp8e4, so the SpMM runs
   in the substituted state w_k := D^{-1} T_k whose recursion is
       w_{k+1} = -2 M^T (D^2 w_k) - w_{k-1}
   The only fp8 rounding is on the matmul input u = fp8(D^2 w) (applied
   per-partition by the ScalarE during the transpose evict).  T_k = D w_k
   is recovered once per conv by scaling the tap-GEMM accumulator with
   dinv (per-partition).  Requires min degree >= 1 and b1 == b2 == 0
   (asserted; true for this problem instance).
 * SpMM matmuls use perf_mode=DoubleRow: stationary = u8 [128, 2, chF]
   (two contraction k-tiles per load), moving = M8 [128, 2, 512].  32
   pair-matmuls x 2 halves replace 64 x 2 fp16 matmuls.
 * AllGather payloads are fp8 (half the bytes) and gather outputs are
   allocated addr_space="Shared" (direct HBM-HBM collective).
 * fc1 weights stream as fp8 (halves the 67MB/core weight read) into an
   fp16 x fp8 matmul; fc1 DMAs ride the idle vector queue and prefetch
   during conv2.
"""

import numpy as np

# ---------------------------------------------------------------- config

class CFG:
    N = 8192          # nodes
    B = 16            # batch
    T = 15            # time taps
    KCH = 25          # chebyshev order
    G1 = 32
    G2 = 64
    C = 512           # fc1 out
    D = 6             # classes
    NCORES = 8
    NCH = 2           # batch chains
    PHASES = 3        # 1=conv1, 2=+conv2, 3=+fc (debug bisect)
    FCW_FP8 = True    # stream fc1 weights as fp8
    DEBUG = False

    @property
    def NLOC(self):
        return self.N // self.NCORES

    @property
    def MT(self):
        return self.NLOC // 128

    @property
    def KT(self):
        return self.N // 128

    @property
    def KTP(self):
        return self.KT // 2          # contraction pairs (32)

    @property
    def BCH(self):
        return self.B // self.NCH        # batches per chain (8)

    @property
    def F1(self):
        return self.BCH * self.T         # conv1 chain width (120)

    @property
    def FW1(self):
        return 128                       # conv1 padded gather width

    @property
    def F2(self):
        return self.BCH * self.G1        # conv2 chain width (256)


def _host_prep(cfg, x, edge_index, W1, b1, W2, b2, fc1_w, fc1_b, fc2_w, fc2_b):
    """Pure layout / format preprocessing -> per-core input maps."""
    import ml_dtypes
    f16 = np.float16
    f8 = ml_dtypes.float8_e4m3
    N, B, T, K = cfg.N, cfg.B, cfg.T, cfg.KCH
    NC, NLOC, MT, KT, KTP = cfg.NCORES, cfg.NLOC, cfg.MT, cfg.KT, cfg.KTP
    G1, G2 = cfg.G1, cfg.G2

    row = np.asarray(edge_index[0], dtype=np.int64)
    col = np.asarray(edge_index[1], dtype=np.int64)
    deg = np.bincount(row, minlength=N).astype(np.float32)
    assert deg.min() >= 1, "w-space substitution needs min degree >= 1"
    assert np.all(np.asarray(b1) == 0) and np.all(np.asarray(b2) == 0), \
        "w-space finalization assumes zero conv biases"
    dinv = (1.0 / np.sqrt(deg)).astype(np.float32)
    # M[r, c] = edge multiplicity;  (Lz)[c] = -dinv_c sum_r M[r,c] dinv_r z_r
    M = np.zeros((N, N), np.float32)
    np.add.at(M, (row, col), 1.0)
    assert M.max() <= 16, "edge multiplicities must stay exactly representable"

    # x node-major fp8, pre-scaled by dinv:  u0 = D x   (n = kt*128 + p)
    xs = np.asarray(x, np.float32) * dinv[None, :, None]
    x_n8 = np.ascontiguousarray(
        xs.transpose(1, 0, 2).reshape(KT, 128, B * T)
        .transpose(1, 0, 2)).astype(f8)

    # fold DFT-real (cosine) matrix into W1:  xf = x @ Cf ; W1f[k] = Cf @ W1[k]
    tt = np.arange(T)
    Cf = np.cos(2 * np.pi * np.outer(tt, tt) / T).astype(np.float32)
    W1f = np.einsum('ts,ksg->ktg', Cf, np.asarray(W1, np.float32))  # [K, T, G1]

    # block-diag over the 8 batches of one chain -> sbuf [F1, K, 256]
    W1blk = np.zeros((K, cfg.F1, cfg.BCH * G1), np.float32)
    for b8 in range(cfg.BCH):
        W1blk[:, b8 * T:(b8 + 1) * T, b8 * G1:(b8 + 1) * G1] = W1f
    W1blk = np.ascontiguousarray(W1blk.transpose(1, 0, 2)).astype(f16)

    # block-diag over 4 batches of one chunk -> sbuf [128, K, 256]
    W2blk = np.zeros((K, 4 * G1, 4 * G2), np.float32)
    for b4 in range(4):
        W2blk[:, b4 * G1:(b4 + 1) * G1, b4 * G2:(b4 + 1) * G2] = \
            np.asarray(W2, np.float32)
    W2blk = np.ascontiguousarray(W2blk.transpose(1, 0, 2)).astype(f16)

    ones_col = np.ones((1, 128), f16)
    fc1b_row = np.asarray(fc1_b, np.float32)[None, :].astype(f16)         # [1, C]
    fc2_wT = np.ascontiguousarray(
        np.asarray(fc2_w, np.float32).T.reshape(cfg.C // 128, 128, cfg.D)
        .transpose(1, 0, 2))                                              # [128, C/128, D] f32
    fc2b_col = np.asarray(fc2_b, np.float32)[None, :]                     # [1, D]
    ones_f32 = np.ones((1, cfg.B), np.float32)

    wv = np.asarray(fc1_w, np.float32).reshape(cfg.C, N, G2)
    xt = np.asarray(x, np.float32).transpose(1, 0, 2)                     # [N, B, T]
    xw = xt / dinv[:, None, None]                                         # w0 = D^{-1} x

    wfc_dt = f8 if cfg.FCW_FP8 else f16

    in_maps = []
    for c in range(NC):
        # M column slice -> [p, pg, jj, m] fp8  (r = (2*pg + jj)*128 + p)
        lt = M[:, c * NLOC:(c + 1) * NLOC]
        lt = np.ascontiguousarray(
            lt.reshape(KTP, 2, 128, NLOC).transpose(2, 0, 1, 3)).astype(f8)
        # local w0 per chain: x_t_w[ch][(b,t), n_loc] fp16
        xl = xw[c * NLOC:(c + 1) * NLOC]                                  # [NLOC, B, T]
        xT = np.ascontiguousarray(
            xl.reshape(NLOC, cfg.NCH, cfg.BCH * T).transpose(1, 2, 0)).astype(f16)
        # per-core node scale vectors [p, mt]
        dloc = dinv[c * NLOC:(c + 1) * NLOC].reshape(MT, 128).T           # [128, MT]
        d2loc = np.ascontiguousarray(dloc * dloc).astype(np.float32)
        dloc = np.ascontiguousarray(dloc).astype(np.float32)
        # fc1 weight slice -> [p, jt, cc] with jt = g*MT + mt, j = jt*128 + p
        ws = wv[:, c * NLOC:(c + 1) * NLOC, :]                            # [C, NLOC, G2]
        ws = ws.reshape(cfg.C, MT, 128, G2).transpose(2, 3, 1, 0)         # [p, g, mt, C]
        ws = np.ascontiguousarray(ws.reshape(128, G2 * MT, cfg.C)).astype(wfc_dt)
        in_maps.append(dict(
            lt=lt, x_n=x_n8, x_t=xT, d2loc=d2loc, dloc=dloc,
            w1blk=W1blk, w2blk=W2blk,
            ones16=ones_col, fc1b=fc1b_row, fc2wt=fc2_wT, fc2b=fc2b_col,
            onesf32=ones_f32, wfc=ws,
        ))
    return in_maps


def _build(cfg):
    import concourse.bass as bass
    import concourse.mybir as mybir
    import concourse.tile as tile
    from concourse import bacc
    from concourse.masks import make_identity

    f8 = mybir.dt.float8e4
    f16 = mybir.dt.float16
    f32 = mybir.dt.float32
    AT = mybir.ActivationFunctionType
    OP = mybir.AluOpType
    AX = mybir.AxisListType
    DR = mybir.MatmulPerfMode.DoubleRow

    N, B, T, K = cfg.N, cfg.B, cfg.T, cfg.KCH
    NC, NLOC, MT, KT, KTP = cfg.NCORES, cfg.NLOC, cfg.MT, cfg.KT, cfg.KTP
    NCH, BCH, F1, F2, FW1 = cfg.NCH, cfg.BCH, cfg.F1, cfg.F2, cfg.FW1
    G1, G2, C, D = cfg.G1, cfg.G2, cfg.C, cfg.D
    RG = [list(range(NC))]
    KTG = KT // 8                       # kt super-tile groups (8); 4 pairs each

    nc = bacc.Bacc("TRN2", target_bir_lowering=False, debug=False,
                   num_devices=NC)

    wfc_dt = f8 if cfg.FCW_FP8 else f16
    dt_in = {
        'lt': ([128, KTP, 2, MT * 128], f8),
        'x_n': ([128, KT, B * T], f8),
        'x_t': ([NCH, F1, NLOC], f16),
        'd2loc': ([128, MT], f32),
        'dloc': ([128, MT], f32),
        'w1blk': ([F1, K, BCH * G1], f16),
        'w2blk': ([4 * G1, K, 4 * G2], f16),
        'ones16': ([1, 128], f16),
        'fc1b': ([1, C], f16),
        'fc2wt': ([128, C // 128, D], f32),
        'fc2b': ([1, D], f32),
        'onesf32': ([1, B], f32),
        'wfc': ([128, G2 * MT, C], wfc_dt),
    }
    din = {k: nc.dram_tensor(k, shp, dt, kind="ExternalInput").ap()
           for k, (shp, dt) in dt_in.items()}
    dout = nc.dram_tensor("out", [B, D], f32, kind="ExternalOutput").ap()
    if cfg.DEBUG:
        dbg_h1 = nc.dram_tensor("dbg_h1", [128, MT, B * G1], f16,
                                kind="ExternalOutput").ap()
        dbg_h2 = nc.dram_tensor("dbg_h2", [128, MT, B * G2], f16,
                                kind="ExternalOutput").ap()

    with tile.TileContext(nc) as tc:
        with (
            tc.tile_pool(name="const", bufs=1) as constp,
            tc.tile_pool(name="dram", bufs=1, space="DRAM") as dramp,
        ):
            # ---------------- constants / persistent state
            LT = constp.tile([128, KTP, 2, MT * 128], f8)
            for g in range(8):
                eng = nc.sync if g % 2 == 0 else nc.gpsimd
                eng.dma_start(LT[:, g * 4:(g + 1) * 4],
                              din['lt'][:, g * 4:(g + 1) * 4])
            ident16 = constp.tile([128, 128], f16)
            make_identity(nc, ident16[:])
            identf32 = constp.tile([32, 32], f32)
            make_identity(nc, identf32[:])
            ones16 = constp.tile([1, 128], f16)
            nc.sync.dma_start(ones16[:], din['ones16'])
            d2sb = constp.tile([128, MT], f32)
            nc.sync.dma_start(d2sb[:], din['d2loc'])
            dinvsb = constp.tile([128, MT], f32)
            nc.sync.dma_start(dinvsb[:], din['dloc'])

            # DRAM gather buffers (2 tap-parity bufs per chain), fp8
            def gbufs(name, fdim):
                gis = [dramp.tile([128, MT * fdim], f8, name=f"{name}i{i}")
                       for i in range(2)]
                gos = [dramp.tile([NC, 128, MT, fdim], f8, name=f"{name}o{i}",
                                  addr_space="Shared")
                       for i in range(2)]
                return gis, gos

            g1 = [gbufs(f"g1c{ch}", FW1) for ch in range(NCH)]
            g2 = [gbufs(f"g2c{ch}", F2) for ch in range(NCH)]
            gh1_i = [dramp.tile([128, MT * F2], f8, name=f"gh1i{ch}")
                     for ch in range(NCH)]
            gh1_o = [dramp.tile([NC, 128, MT, F2], f8, name=f"gh1o{ch}",
                                addr_space="Shared")
                     for ch in range(NCH)]

            # persistent conv state
            accp = tc.tile_pool(name="accp", bufs=1)
            accpp = accp.__enter__()
            acc1 = accpp.tile([128, MT, B * G1], f32)
            w0loc = accpp.tile([128, MT, B * G1], f16)      # relu(acc1) = D^-1 h1
            hD8 = accpp.tile([128, MT, B * G1], f8)         # fp8(D h1)
            acc2 = accpp.tile([128, MT, B * G2], f32)
            h2 = accpp.tile([128, MT, B * G2], f16)         # relu(D acc2)

            # =========================================================
            # generic w-space chebyshev conv driver (fp8 DoubleRow SpMM)
            # =========================================================
            def conv(tag, nch, nq, chF, FW, wblk, zsrc_fn, ztinit_fn,
                     acc, acc_col0_fn, gbuf, pools):
                """One Chebyshev conv in w-space: nch chains x nq chunks.

                wblk: SBUF AP [chF(+), K, 256]; tap k chunk rhs = wblk[:chF, k, :]
                zsrc_fn(kk, ch, g) -> (ap, width): DRAM AP [128, 8, width] fp8
                    = u super-tile (D^2 w_{kk-1} node-major, group g).
                ztinit_fn(ch, q, zt): init zt [chF, NLOC] = chunk of w_0^T.
                acc_col0_fn(ch, q): starting acc column of chunk (ch, q).
                """
                zsp, pszp, pstp, psgp, ztp, curp = pools
                OW = 256
                zt = [[[ztp.tile([chF, NLOC], f16,
                                 name=f"zt{tag}_{ch}_{q}_{par}")
                        for par in range(2)] for q in range(nq)]
                      for ch in range(nch)]
                for ch in range(nch):
                    for q in range(nq):
                        ztinit_fn(ch, q, zt[ch][q][0])

                # k = 0 tap GEMM from the initial state
                for ch in range(nch):
                    for q in range(nq):
                        c0 = acc_col0_fn(ch, q)
                        for m2 in range(MT // 2):
                            pg = psgp.tile([128, 2, OW], f32, tag="pg",
                                           name=f"pg{tag}0_{ch}_{q}_{m2}")
                            for i in range(2):
                                mt = 2 * m2 + i
                                nc.tensor.matmul(
                                    pg[:, i, :],
                                    zt[ch][q][0][:, mt * 128:(mt + 1) * 128],
                                    wblk[:chF, 0, :], start=True, stop=True)
                            nc.vector.tensor_tensor(
                                acc[:, 2 * m2:2 * m2 + 2, c0:c0 + OW],
                                acc[:, 2 * m2:2 * m2 + 2, c0:c0 + OW],
                                pg[:], OP.add)

                for kk in range(1, K):
                    par, prev = kk % 2, (kk - 2) % 2
                    for ch in range(nch):
                        # ---- SpMM: psts[f, c] = sum_r u[r, f] * M[r, c]
                        psts = [pszp.tile([chF, MT * 128], f32, tag="psz",
                                           name=f"psz{tag}_{kk}_{ch}_{q}")
                                for q in range(nq)]
                        for g in range(KTG):
                            src, width = zsrc_fn(kk, ch, g)
                            zs = zsp.tile([128, 8, FW], f8, tag="zs")
                            nc.scalar.dma_start(zs[:, :, 0:width], src)
                            for j4 in range(4):
                                pgi = g * 4 + j4
                                for q in range(nq):
                                    lhs = zs[:, 2 * j4:2 * j4 + 2,
                                             q * chF:(q + 1) * chF]
                                    nc.tensor.matmul(
                                        psts[q][:, 0:512], lhs,
                                        LT[:, pgi, :, 0:512], perf_mode=DR,
                                        start=(pgi == 0), stop=(pgi == KTP - 1))
                                    nc.tensor.matmul(
                                        psts[q][:, 512:1024], lhs,
                                        LT[:, pgi, :, 512:1024], perf_mode=DR,
                                        start=(pgi == 0), stop=(pgi == KTP - 1))
                        # ---- evict: w_1 = -psts ; w_k = -2*psts - w_{k-2}
                        for q in range(nq):
                            dst = zt[ch][q][par]
                            if kk == 1:
                                nc.vector.tensor_scalar_mul(
                                    dst[:], psts[q][:], -1.0)
                            else:
                                nc.vector.scalar_tensor_tensor(
                                    dst[:], psts[q][:], -2.0,
                                    zt[ch][q][prev][:], OP.mult, OP.subtract)
                        # ---- rebuild node-major u8 + gather (skip last tap)
                        if kk < K - 1:
                            cur = curp.tile([128, MT, FW], f8, tag="cur")
                            for q in range(nq):
                                pt = pstp.tile([128, MT, chF], f16, tag="pst")
                                for mt in range(MT):
                                    nc.tensor.transpose(
                                        pt[:, mt, :],
                                        zt[ch][q][par][:, mt * 128:(mt + 1) * 128],
                                        ident16[:chF, :chF])
                                for mt in range(MT):
                                    nc.scalar.activation(
                                        cur[:, mt, q * chF:q * chF + chF],
                                        pt[:, mt, :], AT.Copy,
                                        scale=d2sb[:, mt:mt + 1])
                            gi, go = gbuf[ch][0][kk % 2], gbuf[ch][1][kk % 2]
                            nc.sync.dma_start(
                                gi[:].rearrange("p (m f) -> p m f", m=MT),
                                cur[:])
                            nc.gpsimd.collective_compute(
                                "AllGather", OP.bypass, replica_groups=RG,
                                ins=[gi[:]], outs=[go[:]])
                        # ---- tap GEMMs
                        for q in range(nq):
                            c0 = acc_col0_fn(ch, q)
                            for m2 in range(MT // 2):
                                pg = psgp.tile([128, 2, OW], f32, tag="pg",
                                               name=f"pg{tag}_{kk}_{ch}_{q}_{m2}")
                                for i in range(2):
                                    mt = 2 * m2 + i
                                    nc.tensor.matmul(
                                        pg[:, i, :],
                                        zt[ch][q][par][:, mt * 128:(mt + 1) * 128],
                                        wblk[:chF, kk, :], start=True, stop=True)
                                nc.vector.tensor_tensor(
                                    acc[:, 2 * m2:2 * m2 + 2, c0:c0 + OW],
                                    acc[:, 2 * m2:2 * m2 + 2, c0:c0 + OW],
                                    pg[:], OP.add)

            # =========================================================
            # fc1 weight prefetch pool (idle vector queue; streams during
            # conv2).  Declared early so the first bufs prefetch at t~0.
            # =========================================================
            JT = G2 * MT            # 512 j-tiles
            JBLK = 16
            NBLK = JT // JBLK
            fcw_bufs = 6
            fcwp_cm = tc.tile_pool(name="fcw", bufs=fcw_bufs)
            fcwp = fcwp_cm.__enter__()
            fcw_tiles = []
            for jb in range(NBLK):
                wbuf = fcwp.tile([128, JBLK, C], wfc_dt, tag="wbuf",
                                 name=f"wbuf{jb}")
                nc.vector.dma_start(
                    wbuf[:], din['wfc'][:, jb * JBLK:(jb + 1) * JBLK, :])
                fcw_tiles.append(wbuf)

            # =========================================================
            # conv1: 2 chains x 1 chunk of F1=120
            # =========================================================
            with (
                tc.tile_pool(name="c1sb", bufs=1) as c1sbp,
                tc.tile_pool(name="zs1", bufs=3) as zs1p,
                tc.tile_pool(name="zt1", bufs=1) as zt1p,
                tc.tile_pool(name="cur1", bufs=2) as cur1p,
                tc.tile_pool(name="psz1", bufs=2, space="PSUM") as psz1p,
                tc.tile_pool(name="pst1", bufs=2, space="PSUM") as pst1p,
                tc.tile_pool(name="psg1", bufs=2, space="PSUM") as psg1p,
            ):
                w1 = c1sbp.tile([F1, K, BCH * G1], f16)
                nc.scalar.dma_start(w1[:], din['w1blk'])
                nc.vector.memset(acc1[:], 0.0)

                def zsrc1(kk, ch, g):
                    if kk == 1:   # u0 = D x, node-major from x_n input
                        return (din['x_n'][:, g * 8:(g + 1) * 8,
                                           ch * F1:(ch + 1) * F1], F1)
                    go = g1[ch][1][(kk - 1) % 2]
                    return (go[g], FW1)

                def ztinit1(ch, q, ztile):
                    nc.scalar.dma_start(ztile[:], din['x_t'][ch])

                with nc.named_scope("conv1"):
                    conv("c1", NCH, 1, F1, FW1, w1, zsrc1, ztinit1,
                         acc1, lambda ch, q: ch * (BCH * G1), g1,
                         (zs1p, psz1p, pst1p, psg1p, zt1p, cur1p))

                    # w0 = relu(acc1) ; u0(conv2) = fp8(relu(d2 * acc1))
                    for ch in range(NCH):
                        cs = slice(ch * F2, (ch + 1) * F2)
                        nc.vector.tensor_scalar_max(
                            w0loc[:, :, cs], acc1[:, :, cs], 0.0)
                        for mt in range(MT):
                            nc.scalar.activation(
                                hD8[:, mt, cs], acc1[:, mt, cs], AT.Relu,
                                scale=d2sb[:, mt:mt + 1])
                        nc.sync.dma_start(
                            gh1_i[ch][:].rearrange("p (m f) -> p m f", m=MT),
                            hD8[:, :, cs])
                        nc.gpsimd.collective_compute(
                            "AllGather", OP.bypass, replica_groups=RG,
                            ins=[gh1_i[ch][:]], outs=[gh1_o[ch][:]])
            if cfg.DEBUG:
                nc.sync.dma_start(dbg_h1, w0loc[:])

            if cfg.PHASES < 2:
                zz = constp.tile([B, D], f32)
                nc.vector.memset(zz[:], 0.0)
                nc.sync.dma_start(dout, zz[:])
                accp.__exit__(None, None, None)
                fcwp_cm.__exit__(None, None, None)
                return nc

            # =========================================================
            # conv2: 2 chains x 2 chunks of 128
            # =========================================================
            with (
                tc.tile_pool(name="c2sb", bufs=1) as c2sbp,
                tc.tile_pool(name="zs2", bufs=4) as zs2p,
                tc.tile_pool(name="zt2", bufs=1) as zt2p,
                tc.tile_pool(name="cur2", bufs=2) as cur2p,
                tc.tile_pool(name="psz2", bufs=2, space="PSUM") as psz2p,
                tc.tile_pool(name="pst2", bufs=2, space="PSUM") as pst2p,
                tc.tile_pool(name="psg2", bufs=2, space="PSUM") as psg2p,
            ):
                w2 = c2sbp.tile([4 * G1, K, 4 * G2], f16)
                nc.sync.dma_start(w2[:], din['w2blk'])
                nc.vector.memset(acc2[:], 0.0)

                def zsrc2(kk, ch, g):
                    if kk == 1:   # u0 from the per-chain gather of hD8
                        return (gh1_o[ch][g], F2)
                    go = g2[ch][1][(kk - 1) % 2]
                    return (go[g], F2)

                def ztinit2(ch, q, ztile):
                    # zt = (w0 chunk)^T via PE transposes of w0loc columns
                    f0 = ch * F2 + q * 128
                    pt = pst2p.tile([128, MT, 128], f16, tag="pst")
                    for mt in range(MT):
                        nc.tensor.transpose(
                            pt[:, mt, :], w0loc[:, mt, f0:f0 + 128],
                            ident16[:])
                    nc.vector.tensor_copy(
                        ztile[:].rearrange("p (m f) -> p m f", m=MT), pt[:])

                with nc.named_scope("conv2"):
                    conv("c2", NCH, 2, 128, F2, w2, zsrc2, ztinit2,
                         acc2, lambda ch, q: ch * (BCH * G2) + q * 256, g2,
                         (zs2p, psz2p, pst2p, psg2p, zt2p, cur2p))
                    # h2 = relu(D acc2)
                    for mt in range(MT):
                        nc.scalar.activation(
                            h2[:, mt, :], acc2[:, mt, :], AT.Relu,
                            scale=dinvsb[:, mt:mt + 1])
            if cfg.DEBUG:
                nc.sync.dma_start(dbg_h2, h2[:])

            if cfg.PHASES < 3:
                zz = constp.tile([B, D], f32)
                nc.vector.memset(zz[:], 0.0)
                nc.sync.dma_start(dout, zz[:])
                accp.__exit__(None, None, None)
                fcwp_cm.__exit__(None, None, None)
                return nc

            # =========================================================
            # fc1 (streamed fp8 weights, contraction-sharded) + fc2 + lsm
            # =========================================================
            h2v = h2[:].rearrange("p m (b g) -> p m b g", b=B)
            with (
                nc.named_scope("fc"),
                tc.tile_pool(name="fcps", bufs=1, space="PSUM") as fcpsp,
                tc.tile_pool(name="fcsb", bufs=1) as fcsbp,
                tc.tile_pool(name="fcps2", bufs=2, space="PSUM") as fcps2p,
            ):
                psfc = fcpsp.tile([B, C], f32)
                fc1b_sb = fcsbp.tile([1, C], f16)
                nc.sync.dma_start(fc1b_sb[:], din['fc1b'])
                for jb in range(NBLK):
                    wbuf = fcw_tiles[jb]
                    for ji in range(JBLK):
                        jt = jb * JBLK + ji
                        g, mt = jt // MT, jt % MT
                        nc.tensor.matmul(psfc[:], h2v[:, mt, :, g],
                                         wbuf[:, ji, :],
                                         start=(jt == 0), stop=False)
                nc.tensor.matmul(psfc[:], ones16[:1, :B], fc1b_sb[:1, :],
                                 start=False, stop=True)

                # transpose [B, C] -> [128, C/128, B]
                hsb = fcsbp.tile([B, C], f32)
                nc.vector.tensor_copy(hsb[:], psfc[:])
                hT = fcsbp.tile([128, C // 128, B], f32)
                for t4 in range(C // 128):
                    tp = fcps2p.tile([128, B], f32, tag="fct")
                    nc.tensor.transpose(tp[:], hsb[:, t4 * 128:(t4 + 1) * 128],
                                        identf32[:B, :B])
                    nc.vector.tensor_copy(hT[:, t4, :], tp[:])

                arin = dramp.tile([128, C // 128, B], f32)
                arout = dramp.tile([128, C // 128, B], f32,
                                   addr_space="Shared")
                nc.sync.dma_start(arin[:], hT[:])
                nc.gpsimd.collective_compute(
                    "AllReduce", OP.add, replica_groups=RG,
                    ins=[arin[:]], outs=[arout[:]])
                hTr = fcsbp.tile([128, C // 128, B], f32)
                nc.sync.dma_start(hTr[:], arout[:])

                # fc2: out[d, b] = fc2_w[d, :] @ h[:, b]
                fc2wt = fcsbp.tile([128, C // 128, D], f32)
                nc.sync.dma_start(fc2wt[:], din['fc2wt'])
                fc2b = fcsbp.tile([1, D], f32)
                nc.sync.dma_start(fc2b[:], din['fc2b'])
                onesf32 = fcsbp.tile([1, B], f32)
                nc.sync.dma_start(onesf32[:], din['onesf32'])
                ps2 = fcps2p.tile([B, D], f32, tag="ps2")
                for kt in range(C // 128):
                    nc.tensor.matmul(ps2[:], hTr[:, kt, :], fc2wt[:, kt, :],
                                     start=(kt == 0), stop=False)
                nc.tensor.matmul(ps2[:], onesf32[:1, :], fc2b[:1, :],
                                 start=False, stop=True)
                sm = fcsbp.tile([B, D], f32)
                nc.vector.tensor_copy(sm[:], ps2[:])

                # log_softmax over D (free axis)
                mx = fcsbp.tile([B, 1], f32)
                nc.vector.tensor_reduce(mx[:], sm[:], AX.X, OP.max)
                xm = fcsbp.tile([B, D], f32)
                nc.vector.tensor_single_scalar(xm[:], sm[:], mx[:], OP.subtract)
                ex = fcsbp.tile([B, D], f32)
                nc.scalar.activation(ex[:], xm[:], AT.Exp)
                sume = fcsbp.tile([B, 1], f32)
                nc.vector.tensor_reduce(sume[:], ex[:], AX.X, OP.add)
                lse = fcsbp.tile([B, 1], f32)
                nc.scalar.activation(lse[:], sume[:], AT.Ln)
                res = fcsbp.tile([B, D], f32)
                nc.vector.tensor_single_scalar(res[:], xm[:], lse[:],
                                               OP.subtract)
                nc.sync.dma_start(dout, res[:])
            accp.__exit__(None, None, None)
            fcwp_cm.__exit__(None, None, None)

    return nc


def _run(cfg, inputs, trace=False):
    in_maps = _host_prep(cfg, **inputs)
    nc = _build(cfg)
    nc.compile()
    from concourse import bass_utils
    res = bass_utils.run_bass_kernel_spmd(
        nc, in_maps, core_ids=list(range(cfg.NCORES)), trace=trace)
    return np.asarray(res.results[0]['out'], np.float32).copy(), res


def kernel(**inputs):
    out, _ = _run(CFG(), inputs)
    return out


# revision 16
# speedup vs baseline: 1.1282x; 1.1282x over previous
"""Trainium2 Bass kernel for NetTGCN (gnn_message_passing) — v3.

All Chebyshev SpMMs run in the substituted "w-space":  Lhat = -D M D with
D = diag(dinv) and M the integer edge-multiplicity matrix, and the state
w_k := D^{-1} T_k follows
    w_{k+1} = -2 M^T (D^2 w_k) - w_{k-1}
so the SpMM matrix is M itself — exact in fp16 AND fp8 — and every
rescaling is per-source-node (per-partition ScalarE ops) or a rank-1 bias
(outer(1/dinv, b) PE matmul).  T_k = D w_k is recovered once per conv by
scaling the tap-GEMM accumulator.

Precision/speed plan (validated by CPU simulation against the reference):
 * conv1 (24 taps) and conv2 taps < FP8_FROM run fp16: moving operand is
   M16, column-sharded [N, N/8] (16.8 MB SBUF-resident), v2's
   column-major SpMM shape (stationary u16 tile, 2x512-wide moving).
 * conv2 taps >= FP8_FROM run fp8 DoubleRow: M8 pair-interleaved
   [128, KTP, 2, 1024] (8.4 MB), stationary u8 [128, 2, chF] covering two
   contraction k-tiles per load — half the matmuls at +13% each.  Late
   taps are chosen because their quantization noise propagates through
   the fewest downstream taps (rel-err ~1e-2 vs the 2e-2 gate).
 * M16 and M8 are sequentially resident (M16 released mid-conv2).
 * AllGather payloads are fp8 for the fp8 taps; gather outputs are
   addr_space="Shared" (direct HBM-HBM collective).
 * fc1 weights (67 MB/core fp16) stream through a ring on the idle
   vector queue, prefetching from the M16->M8 swap point onward.
"""

import numpy as np

# ---------------------------------------------------------------- config

class CFG:
    N = 8192          # nodes
    B = 16            # batch
    T = 15            # time taps
    KCH = 25          # chebyshev order
    G1 = 32
    G2 = 64
    C = 512           # fc1 out
    D = 6             # classes
    NCORES = 8
    NCH = 2           # batch chains
    FP8_FROM = 13     # conv2 SpMMs kk >= this use fp8 DoubleRow
    S2 = 0.25         # conv2 w-state prescale (keeps u8 under fp8e4 max)
    FCW_BUFS = 5
    PHASES = 3        # 1=conv1, 2=+conv2, 3=+fc (debug bisect)
    DEBUG = False

    @property
    def NLOC(self):
        return self.N // self.NCORES

    @property
    def MT(self):
        return self.NLOC // 128

    @property
    def KT(self):
        return self.N // 128

    @property
    def KTP(self):
        return self.KT // 2          # contraction pairs (32)

    @property
    def BCH(self):
        return self.B // self.NCH        # batches per chain (8)

    @property
    def F1(self):
        return self.BCH * self.T         # conv1 chain width (120)

    @property
    def F2(self):
        return self.BCH * self.G1        # conv2 chain width (256)


def _host_prep(cfg, x, edge_index, W1, b1, W2, b2, fc1_w, fc1_b, fc2_w, fc2_b):
    """Pure layout / format preprocessing -> per-core input maps."""
    import ml_dtypes
    f16 = np.float16
    f8 = ml_dtypes.float8_e4m3
    N, B, T, K = cfg.N, cfg.B, cfg.T, cfg.KCH
    NC, NLOC, MT, KT, KTP = cfg.NCORES, cfg.NLOC, cfg.MT, cfg.KT, cfg.KTP
    G1, G2 = cfg.G1, cfg.G2

    row = np.asarray(edge_index[0], dtype=np.int64)
    col = np.asarray(edge_index[1], dtype=np.int64)
    deg = np.bincount(row, minlength=N).astype(np.float32)
    assert deg.min() >= 1, "w-space substitution needs min degree >= 1"
    dinv = (1.0 / np.sqrt(deg)).astype(np.float32)
    # M[r, c] = edge multiplicity;  (Lz)[c] = -dinv_c sum_r M[r,c] dinv_r z_r
    M = np.zeros((N, N), np.float32)
    np.add.at(M, (row, col), 1.0)
    assert M.max() <= 16, "edge multiplicities must stay exact in fp8/fp16"

    # u0 = D x -> [p, kt, (b,t)] fp16, node n = kt*128 + p
    xs = np.asarray(x, np.float32) * dinv[None, :, None]
    x_n = np.ascontiguousarray(
        xs.transpose(1, 0, 2).reshape(KT, 128, B * T)
        .transpose(1, 0, 2)).astype(f16)

    # fold DFT-real (cosine) matrix into W1:  xf = x @ Cf ; W1f[k] = Cf @ W1[k]
    tt = np.arange(T)
    Cf = np.cos(2 * np.pi * np.outer(tt, tt) / T).astype(np.float32)
    W1f = np.einsum('ts,ksg->ktg', Cf, np.asarray(W1, np.float32))  # [K, T, G1]

    # block-diag over the 8 batches of one chain -> sbuf [F1, K, 256]
    W1blk = np.zeros((K, cfg.F1, cfg.BCH * G1), np.float32)
    for b8 in range(cfg.BCH):
        W1blk[:, b8 * T:(b8 + 1) * T, b8 * G1:(b8 + 1) * G1] = W1f
    W1blk = np.ascontiguousarray(W1blk.transpose(1, 0, 2)).astype(f16)

    # block-diag over 4 batches of one chunk -> sbuf [128, K, 256].
    # conv2's w-state is globally scaled by S2 to keep u = D^2 w inside
    # fp8e4's +-240 range (device converts overflow to Inf); W2 carries 1/S2.
    S2 = cfg.S2
    W2blk = np.zeros((K, 4 * G1, 4 * G2), np.float32)
    for b4 in range(4):
        W2blk[:, b4 * G1:(b4 + 1) * G1, b4 * G2:(b4 + 1) * G2] = \
            np.asarray(W2, np.float32) / S2
    W2blk = np.ascontiguousarray(W2blk.transpose(1, 0, 2)).astype(f16)

    b1row = np.tile(np.asarray(b1, np.float32), B)[None, :].astype(f16)   # [1, 512]
    b2row = np.tile(np.asarray(b2, np.float32), B)[None, :].astype(f16)   # [1, 1024]
    ones_col = np.ones((1, 128), f16)
    fc1b_row = np.asarray(fc1_b, np.float32)[None, :].astype(f16)         # [1, C]
    fc2_wT = np.ascontiguousarray(
        np.asarray(fc2_w, np.float32).T.reshape(cfg.C // 128, 128, cfg.D)
        .transpose(1, 0, 2))                                              # [128, C/128, D] f32
    fc2b_col = np.asarray(fc2_b, np.float32)[None, :]                     # [1, D]
    ones_f32 = np.ones((1, cfg.B), np.float32)

    wv = np.asarray(fc1_w, np.float32).reshape(cfg.C, N, G2)
    xt = np.asarray(x, np.float32).transpose(1, 0, 2)                     # [N, B, T]
    xw = xt / dinv[:, None, None]                                         # w0 = D^{-1} x

    in_maps = []
    for c in range(NC):
        # M fp16 column slice -> [p, kt, mt, m]  (r = kt*128 + p)
        lt = M[:, c * NLOC:(c + 1) * NLOC]
        lt16 = np.ascontiguousarray(
            lt.reshape(KT, 128, MT, 128).transpose(1, 0, 2, 3)).astype(f16)
        # M fp8 column slice -> [p, pg, jj, m]  (r = (2*pg + jj)*128 + p)
        lt8 = np.ascontiguousarray(
            lt.reshape(KTP, 2, 128, NLOC).transpose(2, 0, 1, 3)).astype(f8)
        # local w0 per chain: x_t[ch][(b,t), n_loc] fp16
        xl = xw[c * NLOC:(c + 1) * NLOC]                                  # [NLOC, B, T]
        xT = np.ascontiguousarray(
            xl.reshape(NLOC, cfg.NCH, cfg.BCH * T).transpose(1, 2, 0)).astype(f16)
        # per-core node scale vectors
        dl = dinv[c * NLOC:(c + 1) * NLOC]
        dloc = np.ascontiguousarray(dl.reshape(MT, 128).T).astype(np.float32)
        d2loc = np.ascontiguousarray(dloc * dloc).astype(np.float32)
        d2loc2 = np.ascontiguousarray(S2 * d2loc).astype(np.float32)
        dinvinvrow = (1.0 / dl)[None, :].astype(f16)                      # [1, NLOC]
        # fc1 weight slice -> [p, jt, cc] with jt = g*MT + mt, j = jt*128 + p
        ws = wv[:, c * NLOC:(c + 1) * NLOC, :]                            # [C, NLOC, G2]
        ws = ws.reshape(cfg.C, MT, 128, G2).transpose(2, 3, 1, 0)         # [p, g, mt, C]
        ws = np.ascontiguousarray(ws.reshape(128, G2 * MT, cfg.C)).astype(f16)
        in_maps.append(dict(
            lt16=lt16, lt8=lt8, x_n=x_n, x_t=xT,
            dloc=dloc, d2loc=d2loc, d2loc2=d2loc2, dinvinvrow=dinvinvrow,
            w1blk=W1blk, w2blk=W2blk, b1row=b1row, b2row=b2row,
            ones16=ones_col, fc1b=fc1b_row, fc2wt=fc2_wT, fc2b=fc2b_col,
            onesf32=ones_f32, wfc=ws,
        ))
    return in_maps


def _build(cfg):
    import concourse.bass as bass
    import concourse.mybir as mybir
    import concourse.tile as tile
    from concourse import bacc
    from concourse.masks import make_identity

    f8 = mybir.dt.float8e4
    f16 = mybir.dt.float16
    f32 = mybir.dt.float32
    AT = mybir.ActivationFunctionType
    OP = mybir.AluOpType
    AX = mybir.AxisListType
    DR = mybir.MatmulPerfMode.DoubleRow

    N, B, T, K = cfg.N, cfg.B, cfg.T, cfg.KCH
    NC, NLOC, MT, KT, KTP = cfg.NCORES, cfg.NLOC, cfg.MT, cfg.KT, cfg.KTP
    NCH, BCH, F1, F2 = cfg.NCH, cfg.BCH, cfg.F1, cfg.F2
    G1, G2, C, D = cfg.G1, cfg.G2, cfg.C, cfg.D
    F8K = cfg.FP8_FROM
    assert 2 <= F8K <= K
    RG = [list(range(NC))]
    KTG = KT // 8                       # kt super-tile groups (8)

    nc = bacc.Bacc("TRN2", target_bir_lowering=False, debug=False,
                   num_devices=NC)

    dt_in = {
        'lt16': ([128, KT, MT, 128], f16),
        'lt8': ([128, KTP, 2, MT * 128], f8),
        'x_n': ([128, KT, B * T], f16),
        'x_t': ([NCH, F1, NLOC], f16),
        'dloc': ([128, MT], f32),
        'd2loc': ([128, MT], f32),
        'd2loc2': ([128, MT], f32),
        'dinvinvrow': ([1, NLOC], f16),
        'w1blk': ([F1, K, BCH * G1], f16),
        'w2blk': ([4 * G1, K, 4 * G2], f16),
        'b1row': ([1, B * G1], f16),
        'b2row': ([1, B * G2], f16),
        'ones16': ([1, 128], f16),
        'fc1b': ([1, C], f16),
        'fc2wt': ([128, C // 128, D], f32),
        'fc2b': ([1, D], f32),
        'onesf32': ([1, B], f32),
        'wfc': ([128, G2 * MT, C], f16),
    }
    din = {k: nc.dram_tensor(k, shp, dt, kind="ExternalInput").ap()
           for k, (shp, dt) in dt_in.items()}
    dout = nc.dram_tensor("out", [B, D], f32, kind="ExternalOutput").ap()
    if cfg.DEBUG:
        dbg_h1 = nc.dram_tensor("dbg_h1", [128, MT, B * G1], f16,
                                kind="ExternalOutput").ap()
        dbg_h2 = nc.dram_tensor("dbg_h2", [128, MT, B * G2], f16,
                                kind="ExternalOutput").ap()

    with tile.TileContext(nc) as tc:
        with (
            tc.tile_pool(name="const", bufs=1) as constp,
            tc.tile_pool(name="dram", bufs=1, space="DRAM") as dramp,
        ):
            # ---------------- constants
            ident16 = constp.tile([128, 128], f16)
            make_identity(nc, ident16[:])
            identf32 = constp.tile([32, 32], f32)
            make_identity(nc, identf32[:])
            ones16 = constp.tile([1, 128], f16)
            nc.sync.dma_start(ones16[:], din['ones16'])
            dinvsb = constp.tile([128, MT], f32)
            nc.sync.dma_start(dinvsb[:], din['dloc'])
            d2sb = constp.tile([128, MT], f32)
            nc.sync.dma_start(d2sb[:], din['d2loc'])
            d2sb2 = constp.tile([128, MT], f32)
            nc.sync.dma_start(d2sb2[:], din['d2loc2'])
            dinvinvrow = constp.tile([1, NLOC], f16)
            nc.sync.dma_start(dinvinvrow[:], din['dinvinvrow'])

            # DRAM gather buffers: gi 2-parity (Local), go per-tap (Shared,
            # write-once) indexed by producing tap kk
            def gbufs(name, fdim, dt, taps):
                gis = [dramp.tile([128, MT * fdim], dt, name=f"{name}i{i}")
                       for i in range(2)]
                gos = {kk: dramp.tile([NC, 128, MT, fdim], dt,
                                      name=f"{name}o{kk}",
                                      addr_space="Shared")
                       for kk in taps}
                return gis, gos

            g1 = [gbufs(f"g1c{ch}", F1, f16, range(1, K - 1))
                  for ch in range(NCH)]
            g2a = [gbufs(f"g2ac{ch}", F2, f16, range(1, F8K - 1))
                   for ch in range(NCH)]
            g2b = [gbufs(f"g2bc{ch}", F2, f8, range(F8K - 1, K - 1))
                   for ch in range(NCH)]
            gh1_i = [dramp.tile([128, MT * F2], f16, name=f"gh1i{ch}")
                     for ch in range(NCH)]
            gh1_o = [dramp.tile([NC, 128, MT, F2], f16, name=f"gh1o{ch}",
                                addr_space="Shared")
                     for ch in range(NCH)]

            # persistent conv state (spans conv1 -> fc)
            accp = tc.tile_pool(name="accp", bufs=1)
            accpp = accp.__enter__()
            acc2 = accpp.tile([128, MT, B * G2], f16)
            zt2 = [[[accpp.tile([128, NLOC], f16, name=f"ztc2_{ch}_{q}_{par}")
                     for par in range(2)] for q in range(2)]
                   for ch in range(NCH)]

            # ---- shared helpers ------------------------------------
            def tapgemm(acc, ztile, wblk, chF, kk, c0, last, brow, psgp, tag):
                for m2 in range(MT // 2):
                    pg = psgp.tile([128, 2, 256], f32, tag="pg",
                                   name=f"pg{tag}_{kk}_{m2}")
                    for i in range(2):
                        mt = 2 * m2 + i
                        nc.tensor.matmul(
                            pg[:, i, :],
                            ztile[:chF, mt * 128:(mt + 1) * 128],
                            wblk[:chF, kk, :], start=True, stop=not last)
                        if last:
                            nc.tensor.matmul(
                                pg[:, i, :],
                                dinvinvrow[:1, mt * 128:(mt + 1) * 128],
                                brow[:1, c0:c0 + 256],
                                start=False, stop=True)
                    nc.vector.tensor_tensor(
                        acc[:, 2 * m2:2 * m2 + 2, c0:c0 + 256],
                        acc[:, 2 * m2:2 * m2 + 2, c0:c0 + 256],
                        pg[:], OP.add)

            def evict(dst, psts, prev, kk):
                if kk == 1:
                    nc.vector.tensor_scalar_mul(dst[:], psts[:], -1.0)
                else:
                    nc.vector.scalar_tensor_tensor(
                        dst[:], psts[:], -2.0, prev[:], OP.mult, OP.subtract)

            def xpose_scale(ztile, chF, cur, qs, pstp, scl):
                # cur[:, mt, qs] = fp( scl * ztile^T )  node-major
                pt = pstp.tile([128, MT, chF], f16, tag="pst")
                for mt in range(MT):
                    nc.tensor.transpose(
                        pt[:, mt, :], ztile[:chF, mt * 128:(mt + 1) * 128],
                        ident16[:chF, :chF])
                for mt in range(MT):
                    nc.vector.tensor_scalar(
                        cur[:, mt, qs], pt[:, mt, :], scl[:, mt:mt + 1],
                        None, OP.mult)

            # =========================================================
            # M16 phase: conv1 (all taps) + conv2 taps < FP8_FROM
            # =========================================================
            m16p_cm = tc.tile_pool(name="m16", bufs=1)
            m16p = m16p_cm.__enter__()
            acc1 = m16p.tile([128, MT, B * G1], f16)
            LT = m16p.tile([128, KT, MT, 128], f16)
            for g in range(8):
                eng = nc.sync if g % 2 == 0 else nc.gpsimd
                eng.dma_start(LT[:, g * 8:(g + 1) * 8],
                              din['lt16'][:, g * 8:(g + 1) * 8])

            def spmm16(psts, zs, chF, q):
                # psts [chF, 1024] accumulates over all KT k-tiles
                for k8 in range(8):
                    lhs = zs[:, k8, q * chF:(q + 1) * chF]
                    kt = spmm16.g * 8 + k8
                    nc.tensor.matmul(psts[:, 0:512], lhs, LT[:, kt, 0:4, :],
                                     start=(kt == 0), stop=(kt == KT - 1))
                    nc.tensor.matmul(psts[:, 512:1024], lhs, LT[:, kt, 4:8, :],
                                     start=(kt == 0), stop=(kt == KT - 1))

            # ---------------- conv1 ----------------
            with (
                tc.tile_pool(name="c1sb", bufs=1) as c1sbp,
                tc.tile_pool(name="zs1", bufs=3) as zs1p,
                tc.tile_pool(name="zt1", bufs=1) as zt1p,
                tc.tile_pool(name="cur1", bufs=2) as cur1p,
                tc.tile_pool(name="psz1", bufs=2, space="PSUM") as psz1p,
                tc.tile_pool(name="pst1", bufs=2, space="PSUM") as pst1p,
                tc.tile_pool(name="psg1", bufs=2, space="PSUM") as psg1p,
            ):
                w1 = c1sbp.tile([F1, K, BCH * G1], f16)
                nc.scalar.dma_start(w1[:], din['w1blk'])
                b1row = c1sbp.tile([1, B * G1], f16)
                nc.scalar.dma_start(b1row[:], din['b1row'])
                nc.vector.memset(acc1[:], 0.0)

                zt1 = [[zt1p.tile([F1, NLOC], f16, name=f"ztc1_{ch}_{par}")
                        for par in range(2)] for ch in range(NCH)]
                for ch in range(NCH):
                    nc.scalar.dma_start(zt1[ch][0][:], din['x_t'][ch])

                with nc.named_scope("conv1"):
                    for ch in range(NCH):
                        tapgemm(acc1, zt1[ch][0], w1, F1, 0,
                                ch * (BCH * G1), False, b1row, psg1p,
                                f"c1_{ch}")
                    for kk in range(1, K):
                        par, prev = kk % 2, (kk - 2) % 2
                        for ch in range(NCH):
                            psts = psz1p.tile([F1, MT * 128], f32, tag="psz",
                                              name=f"pszc1_{kk}_{ch}")
                            for g in range(KTG):
                                if kk == 1:
                                    src = din['x_n'][:, g * 8:(g + 1) * 8,
                                                     ch * F1:(ch + 1) * F1]
                                else:
                                    src = g1[ch][1][kk - 1][g]
                                zs = zs1p.tile([128, 8, F1], f16, tag="zs")
                                nc.scalar.dma_start(zs[:], src)
                                spmm16.g = g
                                spmm16(psts, zs, F1, 0)
                            evict(zt1[ch][par], psts, zt1[ch][prev], kk)
                            if kk < K - 1:
                                cur = cur1p.tile([128, MT, F1], f16,
                                                 tag="cur")
                                xpose_scale(zt1[ch][par], F1, cur,
                                            slice(0, F1), pst1p, d2sb)
                                gi = g1[ch][0][kk % 2]
                                go = g1[ch][1][kk]
                                nc.sync.dma_start(
                                    gi[:].rearrange("p (m f) -> p m f", m=MT),
                                    cur[:])
                                nc.gpsimd.collective_compute(
                                    "AllGather", OP.bypass, replica_groups=RG,
                                    ins=[gi[:]], outs=[go[:]])
                            tapgemm(acc1, zt1[ch][par], w1, F1, kk,
                                    ch * (BCH * G1), kk == K - 1, b1row,
                                    psg1p, f"c1_{ch}")

            # ---------------- conv1 -> conv2 handoff ----------------
            with (
                tc.tile_pool(name="fin", bufs=1) as finp,
                tc.tile_pool(name="pstF", bufs=2, space="PSUM") as pstFp,
            ):
                w0loc = finp.tile([128, MT, B * G1], f16)   # D^{-1} h1
                hD16 = finp.tile([128, MT, B * G1], f16)    # D h1 = u0
                nc.vector.tensor_scalar(w0loc[:], acc1[:], float(cfg.S2),
                                        0.0, OP.mult, OP.max)
                for mt in range(MT):
                    nc.scalar.activation(
                        hD16[:, mt, :], acc1[:, mt, :], AT.Relu,
                        scale=d2sb2[:, mt:mt + 1])
                for ch in range(NCH):
                    cs = slice(ch * F2, (ch + 1) * F2)
                    nc.sync.dma_start(
                        gh1_i[ch][:].rearrange("p (m f) -> p m f", m=MT),
                        hD16[:, :, cs])
                    nc.gpsimd.collective_compute(
                        "AllGather", OP.bypass, replica_groups=RG,
                        ins=[gh1_i[ch][:]], outs=[gh1_o[ch][:]])
                # zt2 init: transpose w0loc chunks
                for ch in range(NCH):
                    for q in range(2):
                        f0 = ch * F2 + q * 128
                        pt = pstFp.tile([128, MT, 128], f16, tag="pst")
                        for mt in range(MT):
                            nc.tensor.transpose(
                                pt[:, mt, :], w0loc[:, mt, f0:f0 + 128],
                                ident16[:])
                        nc.vector.tensor_copy(
                            zt2[ch][q][0][:].rearrange("p (m f) -> p m f",
                                                       m=MT), pt[:])
            if cfg.DEBUG:
                nc.sync.dma_start(dbg_h1, acc1[:])

            if cfg.PHASES < 2:
                zz = constp.tile([B, D], f32)
                nc.vector.memset(zz[:], 0.0)
                nc.sync.dma_start(dout, zz[:])
                m16p_cm.__exit__(None, None, None)
                accp.__exit__(None, None, None)
                return nc

            # ---------------- conv2 taps 0 .. FP8_FROM-1 (fp16) ------
            def conv2_gsets(kk):
                """(consumer set for SpMM kk, producer set for gather kk)."""
                src = g2b if kk >= F8K else g2a
                dst = g2b if (kk + 1) >= F8K else g2a
                return src, dst

            with (
                tc.tile_pool(name="c2asb", bufs=1) as c2asbp,
                tc.tile_pool(name="zs2a", bufs=2) as zs2ap,
                tc.tile_pool(name="cur2a", bufs=2) as cur2ap,
                tc.tile_pool(name="psz2a", bufs=2, space="PSUM") as psz2ap,
                tc.tile_pool(name="pst2a", bufs=2, space="PSUM") as pst2ap,
                tc.tile_pool(name="psg2a", bufs=2, space="PSUM") as psg2ap,
            ):
                w2a = c2asbp.tile([4 * G1, K, 4 * G2], f16)
                nc.sync.dma_start(w2a[:], din['w2blk'])
                b2rowa = c2asbp.tile([1, B * G2], f16)
                nc.sync.dma_start(b2rowa[:], din['b2row'])
                nc.vector.memset(acc2[:], 0.0)

                with nc.named_scope("conv2a"):
                    for ch in range(NCH):
                        for q in range(2):
                            tapgemm(acc2, zt2[ch][q][0], w2a, 128, 0,
                                    ch * (BCH * G2) + q * 256, False,
                                    b2rowa, psg2ap, f"c2a_{ch}_{q}")
                    for kk in range(1, F8K):
                        par, prev = kk % 2, (kk - 2) % 2
                        gsrc, gdst = conv2_gsets(kk)
                        fp8_out = (kk + 1) >= F8K
                        for ch in range(NCH):
                            psts = [psz2ap.tile([128, MT * 128], f32,
                                                tag="psz",
                                                name=f"pszc2a_{kk}_{ch}_{q}")
                                    for q in range(2)]
                            for g in range(KTG):
                                if kk == 1:
                                    src = gh1_o[ch][g]
                                else:
                                    src = gsrc[ch][1][kk - 1][g]
                                zs = zs2ap.tile([128, 8, F2], f16, tag="zs")
                                nc.scalar.dma_start(zs[:], src)
                                spmm16.g = g
                                for q in range(2):
                                    spmm16(psts[q], zs, 128, q)
                            for q in range(2):
                                evict(zt2[ch][q][par], psts[q],
                                      zt2[ch][q][prev], kk)
                            if kk < K - 1:
                                cur = cur2ap.tile(
                                    [128, MT, F2], f8 if fp8_out else f16,
                                    tag="cur8" if fp8_out else "cur",
                                    name=f"cur2a8_{kk}_{ch}" if fp8_out
                                    else None)
                                for q in range(2):
                                    xpose_scale(zt2[ch][q][par], 128, cur,
                                                slice(q * 128, (q + 1) * 128),
                                                pst2ap, d2sb)
                                gi = gdst[ch][0][kk % 2]
                                go = gdst[ch][1][kk]
                                nc.sync.dma_start(
                                    gi[:].rearrange("p (m f) -> p m f", m=MT),
                                    cur[:])
                                nc.gpsimd.collective_compute(
                                    "AllGather", OP.bypass, replica_groups=RG,
                                    ins=[gi[:]], outs=[go[:]])
                            for q in range(2):
                                tapgemm(acc2, zt2[ch][q][par], w2a, 128, kk,
                                        ch * (BCH * G2) + q * 256,
                                        kk == K - 1, b2rowa, psg2ap,
                                        f"c2a_{ch}_{q}")
            m16p_cm.__exit__(None, None, None)

            # =========================================================
            # M8 phase: conv2 taps FP8_FROM .. K-1 (fp8 DoubleRow)
            # =========================================================
            JT = G2 * MT            # 512 j-tiles
            JBLK = 16
            NBLK = JT // JBLK
            fcwp_cm = tc.tile_pool(name="fcw", bufs=cfg.FCW_BUFS)
            fcwp = fcwp_cm.__enter__()

            m8p_cm = tc.tile_pool(name="m8", bufs=1)
            m8p = m8p_cm.__enter__()
            LT8 = m8p.tile([128, KTP, 2, MT * 128], f8)
            for g in range(8):
                eng = nc.sync if g % 2 == 0 else nc.gpsimd
                eng.dma_start(LT8[:, g * 4:(g + 1) * 4],
                              din['lt8'][:, g * 4:(g + 1) * 4])
            fcw_tiles = {}
            for jb in range(cfg.FCW_BUFS):
                wbuf = fcwp.tile([128, JBLK, C], f16, tag="wbuf",
                                 name=f"wbuf{jb}")
                nc.gpsimd.dma_start(
                    wbuf[:], din['wfc'][:, jb * JBLK:(jb + 1) * JBLK, :])
                fcw_tiles[jb] = wbuf

            with (
                tc.tile_pool(name="c2bsb", bufs=1) as c2bsbp,
                tc.tile_pool(name="zs2b", bufs=4) as zs2bp,
                tc.tile_pool(name="cur2b", bufs=2) as cur2bp,
                tc.tile_pool(name="psz2b", bufs=2, space="PSUM") as psz2bp,
                tc.tile_pool(name="pst2b", bufs=2, space="PSUM") as pst2bp,
                tc.tile_pool(name="psg2b", bufs=2, space="PSUM") as psg2bp,
            ):
                w2b = c2bsbp.tile([4 * G1, K, 4 * G2], f16)
                nc.sync.dma_start(w2b[:], din['w2blk'])
                b2rowb = c2bsbp.tile([1, B * G2], f16)
                nc.sync.dma_start(b2rowb[:], din['b2row'])

                with nc.named_scope("conv2b"):
                    for kk in range(F8K, K):
                        par, prev = kk % 2, (kk - 2) % 2
                        gsrc, gdst = conv2_gsets(kk)
                        for ch in range(NCH):
                            psts = [psz2bp.tile([128, MT * 128], f32,
                                                tag="psz",
                                                name=f"pszc2b_{kk}_{ch}_{q}")
                                    for q in range(2)]
                            for g in range(KTG):
                                src = gsrc[ch][1][kk - 1][g]
                                zs = zs2bp.tile([128, 8, F2], f8, tag="zs")
                                nc.scalar.dma_start(zs[:], src)
                                for j4 in range(4):
                                    pgi = g * 4 + j4
                                    for q in range(2):
                                        lhs = zs[:, 2 * j4:2 * j4 + 2,
                                                 q * 128:(q + 1) * 128]
                                        nc.tensor.matmul(
                                            psts[q][:, 0:512], lhs,
                                            LT8[:, pgi, :, 0:512],
                                            perf_mode=DR,
                                            start=(pgi == 0),
                                            stop=(pgi == KTP - 1))
                                        nc.tensor.matmul(
                                            psts[q][:, 512:1024], lhs,
                                            LT8[:, pgi, :, 512:1024],
                                            perf_mode=DR,
                                            start=(pgi == 0),
                                            stop=(pgi == KTP - 1))
                            for q in range(2):
                                evict(zt2[ch][q][par], psts[q],
                                      zt2[ch][q][prev], kk)
                            if kk < K - 1:
                                cur = cur2bp.tile([128, MT, F2], f8,
                                                  tag="cur")
                                for q in range(2):
                                    xpose_scale(zt2[ch][q][par], 128, cur,
                                                slice(q * 128, (q + 1) * 128),
                                                pst2bp, d2sb)
                                gi = gdst[ch][0][kk % 2]
                                go = gdst[ch][1][kk]
                                nc.sync.dma_start(
                                    gi[:].rearrange("p (m f) -> p m f", m=MT),
                                    cur[:])
                                nc.gpsimd.collective_compute(
                                    "AllGather", OP.bypass, replica_groups=RG,
                                    ins=[gi[:]], outs=[go[:]])
                            for q in range(2):
                                tapgemm(acc2, zt2[ch][q][par], w2b, 128, kk,
                                        ch * (BCH * G2) + q * 256,
                                        kk == K - 1, b2rowb, psg2bp,
                                        f"c2b_{ch}_{q}")
                    # h2 = relu(D acc2) in place
                    for mt in range(MT):
                        nc.scalar.activation(
                            acc2[:, mt, :], acc2[:, mt, :], AT.Relu,
                            scale=dinvsb[:, mt:mt + 1])
            m8p_cm.__exit__(None, None, None)
            if cfg.DEBUG:
                nc.sync.dma_start(dbg_h2, acc2[:])

            if cfg.PHASES < 3:
                zz = constp.tile([B, D], f32)
                nc.vector.memset(zz[:], 0.0)
                nc.sync.dma_start(dout, zz[:])
                fcwp_cm.__exit__(None, None, None)
                accp.__exit__(None, None, None)
                return nc

            # =========================================================
            # fc1 (streamed weights, contraction-sharded) + fc2 + lsm
            # =========================================================
            h2v = acc2[:].rearrange("p m (b g) -> p m b g", b=B)
            with (
                nc.named_scope("fc"),
                tc.tile_pool(name="fcps", bufs=1, space="PSUM") as fcpsp,
                tc.tile_pool(name="fcsb", bufs=1) as fcsbp,
                tc.tile_pool(name="fcps2", bufs=2, space="PSUM") as fcps2p,
            ):
                psfc = fcpsp.tile([B, C], f32)
                fc1b_sb = fcsbp.tile([1, C], f16)
                nc.sync.dma_start(fc1b_sb[:], din['fc1b'])
                for jb in range(NBLK):
                    if jb in fcw_tiles:
                        wbuf = fcw_tiles[jb]
                    else:
                        wbuf = fcwp.tile([128, JBLK, C], f16, tag="wbuf",
                                         name=f"wbuf{jb}")
                        nc.gpsimd.dma_start(
                            wbuf[:],
                            din['wfc'][:, jb * JBLK:(jb + 1) * JBLK, :])
                    for ji in range(JBLK):
                        jt = jb * JBLK + ji
                        g, mt = jt // MT, jt % MT
                        nc.tensor.matmul(psfc[:], h2v[:, mt, :, g],
                                         wbuf[:, ji, :],
                                         start=(jt == 0), stop=False)
                nc.tensor.matmul(psfc[:], ones16[:1, :B], fc1b_sb[:1, :],
                                 start=False, stop=True)

                # transpose [B, C] -> [128, C/128, B]
                hsb = fcsbp.tile([B, C], f32)
                nc.vector.tensor_copy(hsb[:], psfc[:])
                hT = fcsbp.tile([128, C // 128, B], f32)
                for t4 in range(C // 128):
                    tp = fcps2p.tile([128, B], f32, tag="fct")
                    nc.tensor.transpose(tp[:], hsb[:, t4 * 128:(t4 + 1) * 128],
                                        identf32[:B, :B])
                    nc.vector.tensor_copy(hT[:, t4, :], tp[:])

                arin = dramp.tile([128, C // 128, B], f32)
                arout = dramp.tile([128, C // 128, B], f32,
                                   addr_space="Shared")
                nc.sync.dma_start(arin[:], hT[:])
                nc.gpsimd.collective_compute(
                    "AllReduce", OP.add, replica_groups=RG,
                    ins=[arin[:]], outs=[arout[:]])
                hTr = fcsbp.tile([128, C // 128, B], f32)
                nc.sync.dma_start(hTr[:], arout[:])

                # fc2: out[d, b] = fc2_w[d, :] @ h[:, b]
                fc2wt = fcsbp.tile([128, C // 128, D], f32)
                nc.sync.dma_start(fc2wt[:], din['fc2wt'])
                fc2b = fcsbp.tile([1, D], f32)
                nc.sync.dma_start(fc2b[:], din['fc2b'])
                onesf32 = fcsbp.tile([1, B], f32)
                nc.sync.dma_start(onesf32[:], din['onesf32'])
                ps2 = fcps2p.tile([B, D], f32, tag="ps2")
                for kt in range(C // 128):
                    nc.tensor.matmul(ps2[:], hTr[:, kt, :], fc2wt[:, kt, :],
                                     start=(kt == 0), stop=False)
                nc.tensor.matmul(ps2[:], onesf32[:1, :], fc2b[:1, :],
                                 start=False, stop=True)
                sm = fcsbp.tile([B, D], f32)
                nc.vector.tensor_copy(sm[:], ps2[:])

                # log_softmax over D (free axis)
                mx = fcsbp.tile([B, 1], f32)
                nc.vector.tensor_reduce(mx[:], sm[:], AX.X, OP.max)
                xm = fcsbp.tile([B, D], f32)
                nc.vector.tensor_single_scalar(xm[:], sm[:], mx[:], OP.subtract)
                ex = fcsbp.tile([B, D], f32)
                nc.scalar.activation(ex[:], xm[:], AT.Exp)
                sume = fcsbp.tile([B, 1], f32)
                nc.vector.tensor_reduce(sume[:], ex[:], AX.X, OP.add)
                lse = fcsbp.tile([B, 1], f32)
                nc.scalar.activation(lse[:], sume[:], AT.Ln)
                res = fcsbp.tile([B, D], f32)
                nc.vector.tensor_single_scalar(res[:], xm[:], lse[:],
                                               OP.subtract)
                nc.sync.dma_start(dout, res[:])
            fcwp_cm.__exit__(None, None, None)
            accp.__exit__(None, None, None)

    return nc


def _run(cfg, inputs, trace=False):
    in_maps = _host_prep(cfg, **inputs)
    nc = _build(cfg)
    nc.compile()
    from concourse import bass_utils
    res = bass_utils.run_bass_kernel_spmd(
        nc, in_maps, core_ids=list(range(cfg.NCORES)), trace=trace)
    return np.asarray(res.results[0]['out'], np.float32).copy(), res


def kernel(**inputs):
    out, _ = _run(CFG(), inputs)
    return out


# revision 17
# speedup vs baseline: 1.1774x; 1.0436x over previous
"""Trainium2 Bass kernel for NetTGCN (gnn_message_passing) — v3.

All Chebyshev SpMMs run in the substituted "w-space":  Lhat = -D M D with
D = diag(dinv) and M the integer edge-multiplicity matrix, and the state
w_k := D^{-1} T_k follows
    w_{k+1} = -2 M^T (D^2 w_k) - w_{k-1}
so the SpMM matrix is M itself — exact in fp16 AND fp8 — and every
rescaling is per-source-node (per-partition ScalarE ops) or a rank-1 bias
(outer(1/dinv, b) PE matmul).  T_k = D w_k is recovered once per conv by
scaling the tap-GEMM accumulator.

Precision/speed plan (validated by CPU simulation against the reference):
 * conv1 (24 taps) and conv2 taps < FP8_FROM run fp16: moving operand is
   M16, column-sharded [N, N/8] (16.8 MB SBUF-resident), v2's
   column-major SpMM shape (stationary u16 tile, 2x512-wide moving).
 * conv2 taps >= FP8_FROM run fp8 DoubleRow: M8 pair-interleaved
   [128, KTP, 2, 1024] (8.4 MB), stationary u8 [128, 2, chF] covering two
   contraction k-tiles per load — half the matmuls at +13% each.  Late
   taps are chosen because their quantization noise propagates through
   the fewest downstream taps (rel-err ~1e-2 vs the 2e-2 gate).
 * M16 and M8 are sequentially resident (M16 released mid-conv2).
 * AllGather payloads are fp8 for the fp8 taps; gather outputs are
   addr_space="Shared" (direct HBM-HBM collective).
 * fc1 weights (67 MB/core fp16) stream through a ring on the idle
   vector queue, prefetching from the M16->M8 swap point onward.
"""

import numpy as np

# ---------------------------------------------------------------- config

class CFG:
    N = 8192          # nodes
    B = 16            # batch
    T = 15            # time taps
    KCH = 25          # chebyshev order
    G1 = 32
    G2 = 64
    C = 512           # fc1 out
    D = 6             # classes
    NCORES = 8
    NCH = 2           # batch chains
    FP8_FROM = 10     # conv2 SpMMs kk >= this use fp8 DoubleRow
    S2 = 0.25         # conv2 w-state prescale (keeps u8 under fp8e4 max)
    FCW_BUFS = 5
    PHASES = 3        # 1=conv1, 2=+conv2, 3=+fc (debug bisect)
    DEBUG = False

    @property
    def NLOC(self):
        return self.N // self.NCORES

    @property
    def MT(self):
        return self.NLOC // 128

    @property
    def KT(self):
        return self.N // 128

    @property
    def KTP(self):
        return self.KT // 2          # contraction pairs (32)

    @property
    def BCH(self):
        return self.B // self.NCH        # batches per chain (8)

    @property
    def F1(self):
        return self.BCH * self.T         # conv1 chain width (120)

    @property
    def F2(self):
        return self.BCH * self.G1        # conv2 chain width (256)


def _host_prep(cfg, x, edge_index, W1, b1, W2, b2, fc1_w, fc1_b, fc2_w, fc2_b):
    """Pure layout / format preprocessing -> per-core input maps."""
    import ml_dtypes
    f16 = np.float16
    f8 = ml_dtypes.float8_e4m3
    N, B, T, K = cfg.N, cfg.B, cfg.T, cfg.KCH
    NC, NLOC, MT, KT, KTP = cfg.NCORES, cfg.NLOC, cfg.MT, cfg.KT, cfg.KTP
    G1, G2 = cfg.G1, cfg.G2

    row = np.asarray(edge_index[0], dtype=np.int64)
    col = np.asarray(edge_index[1], dtype=np.int64)
    deg = np.bincount(row, minlength=N).astype(np.float32)
    assert deg.min() >= 1, "w-space substitution needs min degree >= 1"
    dinv = (1.0 / np.sqrt(deg)).astype(np.float32)
    # M[r, c] = edge multiplicity;  (Lz)[c] = -dinv_c sum_r M[r,c] dinv_r z_r
    M = np.zeros((N, N), np.float32)
    np.add.at(M, (row, col), 1.0)
    assert M.max() <= 16, "edge multiplicities must stay exact in fp8/fp16"

    # u0 = D x -> [p, kt, (b,t)] fp16, node n = kt*128 + p
    xs = np.asarray(x, np.float32) * dinv[None, :, None]
    x_n = np.ascontiguousarray(
        xs.transpose(1, 0, 2).reshape(KT, 128, B * T)
        .transpose(1, 0, 2)).astype(f16)

    # fold DFT-real (cosine) matrix into W1:  xf = x @ Cf ; W1f[k] = Cf @ W1[k]
    tt = np.arange(T)
    Cf = np.cos(2 * np.pi * np.outer(tt, tt) / T).astype(np.float32)
    W1f = np.einsum('ts,ksg->ktg', Cf, np.asarray(W1, np.float32))  # [K, T, G1]

    # block-diag over the 8 batches of one chain -> sbuf [F1, K, 256]
    W1blk = np.zeros((K, cfg.F1, cfg.BCH * G1), np.float32)
    for b8 in range(cfg.BCH):
        W1blk[:, b8 * T:(b8 + 1) * T, b8 * G1:(b8 + 1) * G1] = W1f
    W1blk = np.ascontiguousarray(W1blk.transpose(1, 0, 2)).astype(f16)

    # block-diag over 4 batches of one chunk -> sbuf [128, K, 256].
    # conv2's w-state is globally scaled by S2 to keep u = D^2 w inside
    # fp8e4's +-240 range (device converts overflow to Inf); W2 carries 1/S2.
    S2 = cfg.S2
    W2blk = np.zeros((K, 4 * G1, 4 * G2), np.float32)
    for b4 in range(4):
        W2blk[:, b4 * G1:(b4 + 1) * G1, b4 * G2:(b4 + 1) * G2] = \
            np.asarray(W2, np.float32) / S2
    W2blk = np.ascontiguousarray(W2blk.transpose(1, 0, 2)).astype(f16)

    b1row = np.tile(np.asarray(b1, np.float32), B)[None, :].astype(f16)   # [1, 512]
    b2row = np.tile(np.asarray(b2, np.float32), B)[None, :].astype(f16)   # [1, 1024]
    ones_col = np.ones((1, 128), f16)
    fc1b_row = np.asarray(fc1_b, np.float32)[None, :].astype(f16)         # [1, C]
    fc2_wT = np.ascontiguousarray(
        np.asarray(fc2_w, np.float32).T.reshape(cfg.C // 128, 128, cfg.D)
        .transpose(1, 0, 2))                                              # [128, C/128, D] f32
    fc2b_col = np.asarray(fc2_b, np.float32)[None, :]                     # [1, D]
    ones_f32 = np.ones((1, cfg.B), np.float32)

    wv = np.asarray(fc1_w, np.float32).reshape(cfg.C, N, G2)
    xt = np.asarray(x, np.float32).transpose(1, 0, 2)                     # [N, B, T]
    xw = xt / dinv[:, None, None]                                         # w0 = D^{-1} x

    in_maps = []
    for c in range(NC):
        # M fp16 column slice -> [p, kt, mt, m]  (r = kt*128 + p)
        lt = M[:, c * NLOC:(c + 1) * NLOC]
        lt16 = np.ascontiguousarray(
            lt.reshape(KT, 128, MT, 128).transpose(1, 0, 2, 3)).astype(f16)
        # M fp8 column slice -> [p, pg, jj, m]  (r = (2*pg + jj)*128 + p)
        lt8 = np.ascontiguousarray(
            lt.reshape(KTP, 2, 128, NLOC).transpose(2, 0, 1, 3)).astype(f8)
        # local w0 per chain: x_t[ch][(b,t), n_loc] fp16
        xl = xw[c * NLOC:(c + 1) * NLOC]                                  # [NLOC, B, T]
        xT = np.ascontiguousarray(
            xl.reshape(NLOC, cfg.NCH, cfg.BCH * T).transpose(1, 2, 0)).astype(f16)
        # per-core node scale vectors
        dl = dinv[c * NLOC:(c + 1) * NLOC]
        dloc = np.ascontiguousarray(dl.reshape(MT, 128).T).astype(np.float32)
        d2loc = np.ascontiguousarray(dloc * dloc).astype(np.float32)
        d2loc2 = np.ascontiguousarray(S2 * d2loc).astype(np.float32)
        dinvinvrow = (1.0 / dl)[None, :].astype(f16)                      # [1, NLOC]
        # fc1 weight slice -> [p, jt, cc] with jt = g*MT + mt, j = jt*128 + p
        ws = wv[:, c * NLOC:(c + 1) * NLOC, :]                            # [C, NLOC, G2]
        ws = ws.reshape(cfg.C, MT, 128, G2).transpose(2, 3, 1, 0)         # [p, g, mt, C]
        ws = np.ascontiguousarray(ws.reshape(128, G2 * MT, cfg.C)).astype(f16)
        in_maps.append(dict(
            lt16=lt16, lt8=lt8, x_n=x_n, x_t=xT,
            dloc=dloc, d2loc=d2loc, d2loc2=d2loc2, dinvinvrow=dinvinvrow,
            w1blk=W1blk, w2blk=W2blk, b1row=b1row, b2row=b2row,
            ones16=ones_col, fc1b=fc1b_row, fc2wt=fc2_wT, fc2b=fc2b_col,
            onesf32=ones_f32, wfc=ws,
        ))
    return in_maps


def _build(cfg):
    import concourse.bass as bass
    import concourse.mybir as mybir
    import concourse.tile as tile
    from concourse import bacc
    from concourse.masks import make_identity

    f8 = mybir.dt.float8e4
    f16 = mybir.dt.float16
    f32 = mybir.dt.float32
    AT = mybir.ActivationFunctionType
    OP = mybir.AluOpType
    AX = mybir.AxisListType
    DR = mybir.MatmulPerfMode.DoubleRow

    N, B, T, K = cfg.N, cfg.B, cfg.T, cfg.KCH
    NC, NLOC, MT, KT, KTP = cfg.NCORES, cfg.NLOC, cfg.MT, cfg.KT, cfg.KTP
    NCH, BCH, F1, F2 = cfg.NCH, cfg.BCH, cfg.F1, cfg.F2
    G1, G2, C, D = cfg.G1, cfg.G2, cfg.C, cfg.D
    F8K = cfg.FP8_FROM
    assert 2 <= F8K <= K
    RG = [list(range(NC))]
    KTG = KT // 8                       # kt super-tile groups (8)

    nc = bacc.Bacc("TRN2", target_bir_lowering=False, debug=False,
                   num_devices=NC)

    dt_in = {
        'lt16': ([128, KT, MT, 128], f16),
        'lt8': ([128, KTP, 2, MT * 128], f8),
        'x_n': ([128, KT, B * T], f16),
        'x_t': ([NCH, F1, NLOC], f16),
        'dloc': ([128, MT], f32),
        'd2loc': ([128, MT], f32),
        'd2loc2': ([128, MT], f32),
        'dinvinvrow': ([1, NLOC], f16),
        'w1blk': ([F1, K, BCH * G1], f16),
        'w2blk': ([4 * G1, K, 4 * G2], f16),
        'b1row': ([1, B * G1], f16),
        'b2row': ([1, B * G2], f16),
        'ones16': ([1, 128], f16),
        'fc1b': ([1, C], f16),
        'fc2wt': ([128, C // 128, D], f32),
        'fc2b': ([1, D], f32),
        'onesf32': ([1, B], f32),
        'wfc': ([128, G2 * MT, C], f16),
    }
    din = {k: nc.dram_tensor(k, shp, dt, kind="ExternalInput").ap()
           for k, (shp, dt) in dt_in.items()}
    dout = nc.dram_tensor("out", [B, D], f32, kind="ExternalOutput").ap()
    if cfg.DEBUG:
        dbg_h1 = nc.dram_tensor("dbg_h1", [128, MT, B * G1], f16,
                                kind="ExternalOutput").ap()
        dbg_h2 = nc.dram_tensor("dbg_h2", [128, MT, B * G2], f16,
                                kind="ExternalOutput").ap()

    with tile.TileContext(nc) as tc:
        with (
            tc.tile_pool(name="const", bufs=1) as constp,
            tc.tile_pool(name="dram", bufs=1, space="DRAM") as dramp,
        ):
            # ---------------- constants
            ident16 = constp.tile([128, 128], f16)
            make_identity(nc, ident16[:])
            identf32 = constp.tile([32, 32], f32)
            make_identity(nc, identf32[:])
            ones16 = constp.tile([1, 128], f16)
            nc.sync.dma_start(ones16[:], din['ones16'])
            dinvsb = constp.tile([128, MT], f32)
            nc.sync.dma_start(dinvsb[:], din['dloc'])
            d2sb = constp.tile([128, MT], f32)
            nc.sync.dma_start(d2sb[:], din['d2loc'])
            d2sb2 = constp.tile([128, MT], f32)
            nc.sync.dma_start(d2sb2[:], din['d2loc2'])
            dinvinvrow = constp.tile([1, NLOC], f16)
            nc.sync.dma_start(dinvinvrow[:], din['dinvinvrow'])

            # DRAM gather buffers: gi 2-parity (Local), go per-tap (Shared,
            # write-once) indexed by producing tap kk
            def gbufs(name, fdim, dt, taps):
                gis = [dramp.tile([128, MT * fdim], dt, name=f"{name}i{i}")
                       for i in range(2)]
                gos = {kk: dramp.tile([NC, 128, MT, fdim], dt,
                                      name=f"{name}o{kk}",
                                      addr_space="Shared")
                       for kk in taps}
                return gis, gos

            g1 = [gbufs(f"g1c{ch}", F1, f16, range(1, K - 1))
                  for ch in range(NCH)]
            g2a = [gbufs(f"g2ac{ch}", F2, f16, range(1, F8K - 1))
                   for ch in range(NCH)]
            g2b = [gbufs(f"g2bc{ch}", F2, f8, range(F8K - 1, K - 1))
                   for ch in range(NCH)]
            gh1_i = [dramp.tile([128, MT * F2], f16, name=f"gh1i{ch}")
                     for ch in range(NCH)]
            gh1_o = [dramp.tile([NC, 128, MT, F2], f16, name=f"gh1o{ch}",
                                addr_space="Shared")
                     for ch in range(NCH)]

            # persistent conv state (spans conv1 -> fc)
            accp = tc.tile_pool(name="accp", bufs=1)
            accpp = accp.__enter__()
            acc2 = accpp.tile([128, MT, B * G2], f16)
            zt2 = [[[accpp.tile([128, NLOC], f16, name=f"ztc2_{ch}_{q}_{par}")
                     for par in range(2)] for q in range(2)]
                   for ch in range(NCH)]

            # ---- shared helpers ------------------------------------
            def tapgemm(acc, ztile, wblk, chF, kk, c0, last, brow, psgp, tag):
                for m2 in range(MT // 2):
                    pg = psgp.tile([128, 2, 256], f32, tag="pg",
                                   name=f"pg{tag}_{kk}_{m2}")
                    for i in range(2):
                        mt = 2 * m2 + i
                        nc.tensor.matmul(
                            pg[:, i, :],
                            ztile[:chF, mt * 128:(mt + 1) * 128],
                            wblk[:chF, kk, :], start=True, stop=not last)
                        if last:
                            nc.tensor.matmul(
                                pg[:, i, :],
                                dinvinvrow[:1, mt * 128:(mt + 1) * 128],
                                brow[:1, c0:c0 + 256],
                                start=False, stop=True)
                    nc.vector.tensor_tensor(
                        acc[:, 2 * m2:2 * m2 + 2, c0:c0 + 256],
                        acc[:, 2 * m2:2 * m2 + 2, c0:c0 + 256],
                        pg[:], OP.add)

            def evict(dst, psts, prev, kk):
                if kk == 1:
                    nc.vector.tensor_scalar_mul(dst[:], psts[:], -1.0)
                else:
                    nc.vector.scalar_tensor_tensor(
                        dst[:], psts[:], -2.0, prev[:], OP.mult, OP.subtract)

            def xpose_scale(ztile, chF, cur, qs, pstp, scl):
                # cur[:, mt, qs] = fp( scl * ztile^T )  node-major
                pt = pstp.tile([128, MT, chF], f16, tag="pst")
                for mt in range(MT):
                    nc.tensor.transpose(
                        pt[:, mt, :], ztile[:chF, mt * 128:(mt + 1) * 128],
                        ident16[:chF, :chF])
                for mt in range(MT):
                    nc.vector.tensor_scalar(
                        cur[:, mt, qs], pt[:, mt, :], scl[:, mt:mt + 1],
                        None, OP.mult)

            # =========================================================
            # M16 phase: conv1 (all taps) + conv2 taps < FP8_FROM
            # =========================================================
            m16p_cm = tc.tile_pool(name="m16", bufs=1)
            m16p = m16p_cm.__enter__()
            acc1 = m16p.tile([128, MT, B * G1], f16)
            LT = m16p.tile([128, KT, MT, 128], f16)
            for g in range(8):
                eng = nc.sync if g % 2 == 0 else nc.gpsimd
                eng.dma_start(LT[:, g * 8:(g + 1) * 8],
                              din['lt16'][:, g * 8:(g + 1) * 8])

            def spmm16(psts, zs, chF, q):
                # psts [chF, 1024] accumulates over all KT k-tiles
                for k8 in range(8):
                    lhs = zs[:, k8, q * chF:(q + 1) * chF]
                    kt = spmm16.g * 8 + k8
                    nc.tensor.matmul(psts[:, 0:512], lhs, LT[:, kt, 0:4, :],
                                     start=(kt == 0), stop=(kt == KT - 1))
                    nc.tensor.matmul(psts[:, 512:1024], lhs, LT[:, kt, 4:8, :],
                                     start=(kt == 0), stop=(kt == KT - 1))

            # ---------------- conv1 ----------------
            with (
                tc.tile_pool(name="c1sb", bufs=1) as c1sbp,
                tc.tile_pool(name="zs1", bufs=3) as zs1p,
                tc.tile_pool(name="zt1", bufs=1) as zt1p,
                tc.tile_pool(name="cur1", bufs=2) as cur1p,
                tc.tile_pool(name="psz1", bufs=2, space="PSUM") as psz1p,
                tc.tile_pool(name="pst1", bufs=2, space="PSUM") as pst1p,
                tc.tile_pool(name="psg1", bufs=2, space="PSUM") as psg1p,
            ):
                w1 = c1sbp.tile([F1, K, BCH * G1], f16)
                nc.scalar.dma_start(w1[:], din['w1blk'])
                b1row = c1sbp.tile([1, B * G1], f16)
                nc.scalar.dma_start(b1row[:], din['b1row'])
                nc.vector.memset(acc1[:], 0.0)

                zt1 = [[zt1p.tile([F1, NLOC], f16, name=f"ztc1_{ch}_{par}")
                        for par in range(2)] for ch in range(NCH)]
                for ch in range(NCH):
                    nc.scalar.dma_start(zt1[ch][0][:], din['x_t'][ch])

                with nc.named_scope("conv1"):
                    for ch in range(NCH):
                        tapgemm(acc1, zt1[ch][0], w1, F1, 0,
                                ch * (BCH * G1), False, b1row, psg1p,
                                f"c1_{ch}")
                    for kk in range(1, K):
                        par, prev = kk % 2, (kk - 2) % 2
                        for ch in range(NCH):
                            psts = psz1p.tile([F1, MT * 128], f32, tag="psz",
                                              name=f"pszc1_{kk}_{ch}")
                            for g in range(KTG):
                                if kk == 1:
                                    src = din['x_n'][:, g * 8:(g + 1) * 8,
                                                     ch * F1:(ch + 1) * F1]
                                else:
                                    src = g1[ch][1][kk - 1][g]
                                zs = zs1p.tile([128, 8, F1], f16, tag="zs")
                                nc.scalar.dma_start(zs[:], src)
                                spmm16.g = g
                                spmm16(psts, zs, F1, 0)
                            evict(zt1[ch][par], psts, zt1[ch][prev], kk)
                            if kk < K - 1:
                                cur = cur1p.tile([128, MT, F1], f16,
                                                 tag="cur")
                                xpose_scale(zt1[ch][par], F1, cur,
                                            slice(0, F1), pst1p, d2sb)
                                gi = g1[ch][0][kk % 2]
                                go = g1[ch][1][kk]
                                nc.sync.dma_start(
                                    gi[:].rearrange("p (m f) -> p m f", m=MT),
                                    cur[:])
                                nc.gpsimd.collective_compute(
                                    "AllGather", OP.bypass, replica_groups=RG,
                                    ins=[gi[:]], outs=[go[:]])
                            tapgemm(acc1, zt1[ch][par], w1, F1, kk,
                                    ch * (BCH * G1), kk == K - 1, b1row,
                                    psg1p, f"c1_{ch}")

            # ---------------- conv1 -> conv2 handoff ----------------
            with (
                tc.tile_pool(name="fin", bufs=1) as finp,
                tc.tile_pool(name="pstF", bufs=2, space="PSUM") as pstFp,
            ):
                w0loc = finp.tile([128, MT, B * G1], f16)   # D^{-1} h1
                hD16 = finp.tile([128, MT, B * G1], f16)    # D h1 = u0
                for ch in range(NCH):
                    cs = slice(ch * F2, (ch + 1) * F2)
                    nc.vector.tensor_scalar(
                        w0loc[:, :, cs], acc1[:, :, cs], float(cfg.S2),
                        0.0, OP.mult, OP.max)
                    for mt in range(MT):
                        nc.scalar.activation(
                            hD16[:, mt, cs], acc1[:, mt, cs], AT.Relu,
                            scale=d2sb2[:, mt:mt + 1])
                    nc.sync.dma_start(
                        gh1_i[ch][:].rearrange("p (m f) -> p m f", m=MT),
                        hD16[:, :, cs])
                    nc.gpsimd.collective_compute(
                        "AllGather", OP.bypass, replica_groups=RG,
                        ins=[gh1_i[ch][:]], outs=[gh1_o[ch][:]])
                    for q in range(2):
                        f0 = ch * F2 + q * 128
                        pt = pstFp.tile([128, MT, 128], f16, tag="pst")
                        for mt in range(MT):
                            nc.tensor.transpose(
                                pt[:, mt, :], w0loc[:, mt, f0:f0 + 128],
                                ident16[:])
                        nc.vector.tensor_copy(
                            zt2[ch][q][0][:].rearrange("p (m f) -> p m f",
                                                       m=MT), pt[:])
            if cfg.DEBUG:
                nc.sync.dma_start(dbg_h1, acc1[:])

            if cfg.PHASES < 2:
                zz = constp.tile([B, D], f32)
                nc.vector.memset(zz[:], 0.0)
                nc.sync.dma_start(dout, zz[:])
                m16p_cm.__exit__(None, None, None)
                accp.__exit__(None, None, None)
                return nc

            # ---------------- conv2 taps 0 .. FP8_FROM-1 (fp16) ------
            def conv2_gsets(kk):
                """(consumer set for SpMM kk, producer set for gather kk)."""
                src = g2b if kk >= F8K else g2a
                dst = g2b if (kk + 1) >= F8K else g2a
                return src, dst

            with (
                tc.tile_pool(name="c2asb", bufs=1) as c2asbp,
                tc.tile_pool(name="zs2a", bufs=2) as zs2ap,
                tc.tile_pool(name="cur2a", bufs=2) as cur2ap,
                tc.tile_pool(name="psz2a", bufs=2, space="PSUM") as psz2ap,
                tc.tile_pool(name="pst2a", bufs=2, space="PSUM") as pst2ap,
                tc.tile_pool(name="psg2a", bufs=2, space="PSUM") as psg2ap,
            ):
                w2a = c2asbp.tile([4 * G1, K, 4 * G2], f16)
                nc.sync.dma_start(w2a[:], din['w2blk'])
                b2rowa = c2asbp.tile([1, B * G2], f16)
                nc.sync.dma_start(b2rowa[:], din['b2row'])
                nc.vector.memset(acc2[:], 0.0)

                with nc.named_scope("conv2a"):
                    for ch in range(NCH):
                        for q in range(2):
                            tapgemm(acc2, zt2[ch][q][0], w2a, 128, 0,
                                    ch * (BCH * G2) + q * 256, False,
                                    b2rowa, psg2ap, f"c2a_{ch}_{q}")
                    for kk in range(1, F8K):
                        par, prev = kk % 2, (kk - 2) % 2
                        gsrc, gdst = conv2_gsets(kk)
                        fp8_out = (kk + 1) >= F8K
                        for ch in range(NCH):
                            psts = [psz2ap.tile([128, MT * 128], f32,
                                                tag="psz",
                                                name=f"pszc2a_{kk}_{ch}_{q}")
                                    for q in range(2)]
                            for g in range(KTG):
                                if kk == 1:
                                    src = gh1_o[ch][g]
                                else:
                                    src = gsrc[ch][1][kk - 1][g]
                                zs = zs2ap.tile([128, 8, F2], f16, tag="zs")
                                nc.scalar.dma_start(zs[:], src)
                                spmm16.g = g
                                for q in range(2):
                                    spmm16(psts[q], zs, 128, q)
                            for q in range(2):
                                evict(zt2[ch][q][par], psts[q],
                                      zt2[ch][q][prev], kk)
                            if kk < K - 1:
                                cur = cur2ap.tile(
                                    [128, MT, F2], f8 if fp8_out else f16,
                                    tag="cur8" if fp8_out else "cur",
                                    name=f"cur2a8_{kk}_{ch}" if fp8_out
                                    else None)
                                for q in range(2):
                                    xpose_scale(zt2[ch][q][par], 128, cur,
                                                slice(q * 128, (q + 1) * 128),
                                                pst2ap, d2sb)
                                gi = gdst[ch][0][kk % 2]
                                go = gdst[ch][1][kk]
                                nc.sync.dma_start(
                                    gi[:].rearrange("p (m f) -> p m f", m=MT),
                                    cur[:])
                                nc.gpsimd.collective_compute(
                                    "AllGather", OP.bypass, replica_groups=RG,
                                    ins=[gi[:]], outs=[go[:]])
                            for q in range(2):
                                tapgemm(acc2, zt2[ch][q][par], w2a, 128, kk,
                                        ch * (BCH * G2) + q * 256,
                                        kk == K - 1, b2rowa, psg2ap,
                                        f"c2a_{ch}_{q}")
            m16p_cm.__exit__(None, None, None)

            # =========================================================
            # M8 phase: conv2 taps FP8_FROM .. K-1 (fp8 DoubleRow)
            # =========================================================
            JT = G2 * MT            # 512 j-tiles
            JBLK = 16
            NBLK = JT // JBLK
            fcwp_cm = tc.tile_pool(name="fcw", bufs=cfg.FCW_BUFS)
            fcwp = fcwp_cm.__enter__()

            m8p_cm = tc.tile_pool(name="m8", bufs=1)
            m8p = m8p_cm.__enter__()
            LT8 = m8p.tile([128, KTP, 2, MT * 128], f8)
            for g in range(8):
                eng = nc.sync if g % 2 == 0 else nc.gpsimd
                eng.dma_start(LT8[:, g * 4:(g + 1) * 4],
                              din['lt8'][:, g * 4:(g + 1) * 4])
            fcw_tiles = {}
            for jb in range(cfg.FCW_BUFS):
                wbuf = fcwp.tile([128, JBLK, C], f16, tag="wbuf",
                                 name=f"wbuf{jb}")
                nc.gpsimd.dma_start(
                    wbuf[:], din['wfc'][:, jb * JBLK:(jb + 1) * JBLK, :])
                fcw_tiles[jb] = wbuf

            with (
                tc.tile_pool(name="c2bsb", bufs=1) as c2bsbp,
                tc.tile_pool(name="zs2b", bufs=4) as zs2bp,
                tc.tile_pool(name="cur2b", bufs=2) as cur2bp,
                tc.tile_pool(name="psz2b", bufs=2, space="PSUM") as psz2bp,
                tc.tile_pool(name="pst2b", bufs=2, space="PSUM") as pst2bp,
                tc.tile_pool(name="psg2b", bufs=2, space="PSUM") as psg2bp,
            ):
                w2b = c2bsbp.tile([4 * G1, K, 4 * G2], f16)
                nc.sync.dma_start(w2b[:], din['w2blk'])
                b2rowb = c2bsbp.tile([1, B * G2], f16)
                nc.sync.dma_start(b2rowb[:], din['b2row'])

                with nc.named_scope("conv2b"):
                    for kk in range(F8K, K):
                        par, prev = kk % 2, (kk - 2) % 2
                        gsrc, gdst = conv2_gsets(kk)
                        for ch in range(NCH):
                            psts = [psz2bp.tile([128, MT * 128], f32,
                                                tag="psz",
                                                name=f"pszc2b_{kk}_{ch}_{q}")
                                    for q in range(2)]
                            for g in range(KTG):
                                src = gsrc[ch][1][kk - 1][g]
                                zs = zs2bp.tile([128, 8, F2], f8, tag="zs")
                                nc.scalar.dma_start(zs[:], src)
                                for j4 in range(4):
                                    pgi = g * 4 + j4
                                    for q in range(2):
                                        lhs = zs[:, 2 * j4:2 * j4 + 2,
                                                 q * 128:(q + 1) * 128]
                                        nc.tensor.matmul(
                                            psts[q][:, 0:512], lhs,
                                            LT8[:, pgi, :, 0:512],
                                            perf_mode=DR,
                                            start=(pgi == 0),
                                            stop=(pgi == KTP - 1))
                                        nc.tensor.matmul(
                                            psts[q][:, 512:1024], lhs,
                                            LT8[:, pgi, :, 512:1024],
                                            perf_mode=DR,
                                            start=(pgi == 0),
                                            stop=(pgi == KTP - 1))
                            for q in range(2):
                                evict(zt2[ch][q][par], psts[q],
                                      zt2[ch][q][prev], kk)
                            if kk < K - 1:
                                cur = cur2bp.tile([128, MT, F2], f8,
                                                  tag="cur")
                                for q in range(2):
                                    xpose_scale(zt2[ch][q][par], 128, cur,
                                                slice(q * 128, (q + 1) * 128),
                                                pst2bp, d2sb)
                                gi = gdst[ch][0][kk % 2]
                                go = gdst[ch][1][kk]
                                nc.sync.dma_start(
                                    gi[:].rearrange("p (m f) -> p m f", m=MT),
                                    cur[:])
                                nc.gpsimd.collective_compute(
                                    "AllGather", OP.bypass, replica_groups=RG,
                                    ins=[gi[:]], outs=[go[:]])
                            for q in range(2):
                                tapgemm(acc2, zt2[ch][q][par], w2b, 128, kk,
                                        ch * (BCH * G2) + q * 256,
                                        kk == K - 1, b2rowb, psg2bp,
                                        f"c2b_{ch}_{q}")
                    # h2 = relu(D acc2) in place
                    for mt in range(MT):
                        nc.scalar.activation(
                            acc2[:, mt, :], acc2[:, mt, :], AT.Relu,
                            scale=dinvsb[:, mt:mt + 1])
            m8p_cm.__exit__(None, None, None)
            if cfg.DEBUG:
                nc.sync.dma_start(dbg_h2, acc2[:])

            if cfg.PHASES < 3:
                zz = constp.tile([B, D], f32)
                nc.vector.memset(zz[:], 0.0)
                nc.sync.dma_start(dout, zz[:])
                fcwp_cm.__exit__(None, None, None)
                accp.__exit__(None, None, None)
                return nc

            # =========================================================
            # fc1 (streamed weights, contraction-sharded) + fc2 + lsm
            # =========================================================
            h2v = acc2[:].rearrange("p m (b g) -> p m b g", b=B)
            with (
                nc.named_scope("fc"),
                tc.tile_pool(name="fcps", bufs=1, space="PSUM") as fcpsp,
                tc.tile_pool(name="fcsb", bufs=1) as fcsbp,
                tc.tile_pool(name="fcps2", bufs=2, space="PSUM") as fcps2p,
            ):
                psfc = fcpsp.tile([B, C], f32)
                fc1b_sb = fcsbp.tile([1, C], f16)
                nc.sync.dma_start(fc1b_sb[:], din['fc1b'])
                for jb in range(NBLK):
                    if jb in fcw_tiles:
                        wbuf = fcw_tiles[jb]
                    else:
                        wbuf = fcwp.tile([128, JBLK, C], f16, tag="wbuf",
                                         name=f"wbuf{jb}")
                        nc.gpsimd.dma_start(
                            wbuf[:],
                            din['wfc'][:, jb * JBLK:(jb + 1) * JBLK, :])
                    for ji in range(JBLK):
                        jt = jb * JBLK + ji
                        g, mt = jt // MT, jt % MT
                        nc.tensor.matmul(psfc[:], h2v[:, mt, :, g],
                                         wbuf[:, ji, :],
                                         start=(jt == 0), stop=False)
                nc.tensor.matmul(psfc[:], ones16[:1, :B], fc1b_sb[:1, :],
                                 start=False, stop=True)

                # transpose [B, C] -> [128, C/128, B]
                hsb = fcsbp.tile([B, C], f32)
                nc.vector.tensor_copy(hsb[:], psfc[:])
                hT = fcsbp.tile([128, C // 128, B], f32)
                for t4 in range(C // 128):
                    tp = fcps2p.tile([128, B], f32, tag="fct")
                    nc.tensor.transpose(tp[:], hsb[:, t4 * 128:(t4 + 1) * 128],
                                        identf32[:B, :B])
                    nc.vector.tensor_copy(hT[:, t4, :], tp[:])

                arin = dramp.tile([128, C // 128, B], f32)
                arout = dramp.tile([128, C // 128, B], f32,
                                   addr_space="Shared")
                nc.sync.dma_start(arin[:], hT[:])
                nc.gpsimd.collective_compute(
                    "AllReduce", OP.add, replica_groups=RG,
                    ins=[arin[:]], outs=[arout[:]])
                hTr = fcsbp.tile([128, C // 128, B], f32)
                nc.sync.dma_start(hTr[:], arout[:])

                # fc2: out[d, b] = fc2_w[d, :] @ h[:, b]
                fc2wt = fcsbp.tile([128, C // 128, D], f32)
                nc.sync.dma_start(fc2wt[:], din['fc2wt'])
                fc2b = fcsbp.tile([1, D], f32)
                nc.sync.dma_start(fc2b[:], din['fc2b'])
                onesf32 = fcsbp.tile([1, B], f32)
                nc.sync.dma_start(onesf32[:], din['onesf32'])
                ps2 = fcps2p.tile([B, D], f32, tag="ps2")
                for kt in range(C // 128):
                    nc.tensor.matmul(ps2[:], hTr[:, kt, :], fc2wt[:, kt, :],
                                     start=(kt == 0), stop=False)
                nc.tensor.matmul(ps2[:], onesf32[:1, :], fc2b[:1, :],
                                 start=False, stop=True)
                sm = fcsbp.tile([B, D], f32)
                nc.vector.tensor_copy(sm[:], ps2[:])

                # log_softmax over D (free axis)
                mx = fcsbp.tile([B, 1], f32)
                nc.vector.tensor_reduce(mx[:], sm[:], AX.X, OP.max)
                xm = fcsbp.tile([B, D], f32)
                nc.vector.tensor_single_scalar(xm[:], sm[:], mx[:], OP.subtract)
                ex = fcsbp.tile([B, D], f32)
                nc.scalar.activation(ex[:], xm[:], AT.Exp)
                sume = fcsbp.tile([B, 1], f32)
                nc.vector.tensor_reduce(sume[:], ex[:], AX.X, OP.add)
                lse = fcsbp.tile([B, 1], f32)
                nc.scalar.activation(lse[:], sume[:], AT.Ln)
                res = fcsbp.tile([B, D], f32)
                nc.vector.tensor_single_scalar(res[:], xm[:], lse[:],
                                               OP.subtract)
                nc.sync.dma_start(dout, res[:])
            fcwp_cm.__exit__(None, None, None)
            accp.__exit__(None, None, None)

    return nc


def _run(cfg, inputs, trace=False):
    in_maps = _host_prep(cfg, **inputs)
    nc = _build(cfg)
    nc.compile()
    from concourse import bass_utils
    res = bass_utils.run_bass_kernel_spmd(
        nc, in_maps, core_ids=list(range(cfg.NCORES)), trace=trace)
    return np.asarray(res.results[0]['out'], np.float32).copy(), res


def kernel(**inputs):
    out, _ = _run(CFG(), inputs)
    return out
